# revision 1
# baseline (speedup 1.0000x reference)
"""2-layer GCN (PyG GCNConv style) on 8 Trainium2 NeuronCores.

Strategy (graph/node parallel, per sharding hint):
  - Nodes are range-sharded across 8 cores (R = N/8 rows each).
  - Host precomputes per-edge normalization (deg^-1/2 products, incl.
    self-loops) and packs each core's incoming edges into fixed-size
    [128, ntiles*C] SBUF-layout arrays (tile = 128 destination nodes,
    C chunks of 128 edges per tile, padded with src=0 / norm=0).
  - Device per core:
      phase 1: xw1 = x_c @ W1 (own rows)           -> AllGather full xw1
      phase 2: per dst-tile: indirect-DMA gather xw1[src] rows, build
               selection matrix S[e,d] = (dstl[e]==d)*norm[e] on DVE,
               PE-matmul-accumulate  G.T @ S  (feature-major output),
               ReLU+bias on PSUM->SBUF copy  => h1^T kept in SBUF
      phase 3: hw2 = h1 @ W2 (uses h1^T as lhsT)   -> AllGather full hw2
      phase 4: same aggregation, node-major out = S.T @ G2, bias via
               rank-1 ones@b2 matmul, DMA to output.
  - Host concatenates the 8 row-shards.
"""

import sys

for p in ("/opt/trn_rl_repo",):
    if p not in sys.path:
        sys.path.insert(0, p)

import numpy as np

import concourse.bass as bass
import concourse.bacc as bacc
import concourse.mybir as mybir
import concourse.tile as tile
from concourse import bass_utils
from concourse.masks import make_identity

P = 128
NCORES = 8


# ----------------------------------------------------------------------------
# Host-side preprocessing
# ----------------------------------------------------------------------------

def _preprocess(x, edge_index, n_cores):
    """Returns per-core packed edge arrays + x shards (transposed)."""
    N = x.shape[0]
    R = N // n_cores
    assert R * n_cores == N
    ntiles = (R + P - 1) // P

    src = edge_index[0].astype(np.int64)
    dst = edge_index[1].astype(np.int64)
    loops = np.arange(N, dtype=np.int64)
    src = np.concatenate([src, loops])
    dst = np.concatenate([dst, loops])

    deg = np.bincount(dst, minlength=N).astype(np.float32)
    dis = np.where(deg > 0, 1.0 / np.sqrt(deg), 0.0).astype(np.float32)
    norm = (dis[src] * dis[dst]).astype(np.float32)

    # global max chunk count C
    tile_g = dst // P  # global 128-node tile id (tiles aligned with cores since R%P may != 0 -- NO)
    # tiles are per-core relative; compute per-core tile ids
    core_id = dst // R
    dloc = dst - core_id * R
    tl = dloc // P
    dstl = (dloc - tl * P).astype(np.float32)

    counts = np.bincount(core_id * ntiles + tl,
                         minlength=n_cores * ntiles).reshape(n_cores, ntiles)
    # per-tile-slot chunk count = max over cores (SPMD program is shared)
    chunks = [int(x) for x in
              np.ceil(counts.max(axis=0) / P).astype(np.int64)]
    total = int(sum(chunks))
    offs = np.concatenate([[0], np.cumsum(chunks)])  # in chunks

    packed = []
    for c in range(n_cores):
        m = core_id == c
        s_c = src[m].astype(np.int32)
        t_c = tl[m]
        d_c = dstl[m]
        n_c = norm[m]
        order = np.argsort(t_c, kind="stable")
        s_c, t_c, d_c, n_c = s_c[order], t_c[order], d_c[order], n_c[order]
        cnt = np.bincount(t_c, minlength=ntiles)
        starts = np.cumsum(cnt) - cnt
        pos = np.arange(len(t_c)) - np.repeat(starts, cnt)
        slots = offs[t_c] * P + pos
        A_src = np.zeros(total * P, np.int32)
        A_dl = np.zeros(total * P, np.float32)
        A_nm = np.zeros(total * P, np.float32)
        A_src[slots] = s_c
        A_dl[slots] = d_c
        A_nm[slots] = n_c

        def lay(a):
            return np.ascontiguousarray(
                a.reshape(total, P).T)

        packed.append((lay(A_src), lay(A_dl), lay(A_nm)))
    return packed, chunks, R, ntiles


# ----------------------------------------------------------------------------
# Device kernel builder (parameterized so a tiny config can be sim-tested)
# ----------------------------------------------------------------------------

def build_nc(N, R, ntiles, chunks, F0, F1, F2, n_cores):
    """Build the SPMD Bass program. All dims: F0,F1,F2 multiples of 128."""
    f32 = mybir.dt.float32
    i32 = mybir.dt.int32
    K0 = F0 // P       # k-tiles in layer-1 matmul
    H1 = F1 // P       # 128-wide halves of F1
    K2 = F1 // P       # k-tiles in layer-2 matmul (= H1)
    assert F2 <= 512 and F2 % P == 0
    last_rows = R - (ntiles - 1) * P  # rows in the final (possibly partial) tile
    RP = ntiles * P    # padded row count
    total = int(sum(chunks))
    offs = [0]
    for c in chunks:
        offs.append(offs[-1] + c)

    nc = bacc.Bacc("TRN2", target_bir_lowering=False, debug=False,
                   num_devices=n_cores)

    xT = nc.dram_tensor("xT", [F0, R], f32, kind="ExternalInput").ap()
    srcs_d = nc.dram_tensor("srcs", [P, total], i32, kind="ExternalInput").ap()
    dstl_d = nc.dram_tensor("dstl", [P, total], f32, kind="ExternalInput").ap()
    nrm_d = nc.dram_tensor("nrm", [P, total], f32, kind="ExternalInput").ap()
    W1_d = nc.dram_tensor("W1", [F0, F1], f32, kind="ExternalInput").ap()
    b1_d = nc.dram_tensor("b1", [F1], f32, kind="ExternalInput").ap()
    W2_d = nc.dram_tensor("W2", [F1, F2], f32, kind="ExternalInput").ap()
    b2_d = nc.dram_tensor("b2", [F2], f32, kind="ExternalInput").ap()
    out_d = nc.dram_tensor("out", [R, F2], f32, kind="ExternalOutput").ap()

    rg = [list(range(n_cores))]

    with tile.TileContext(nc) as tc:
        with (
            tc.tile_pool(name="dram", bufs=1, space="DRAM") as dram,
            tc.tile_pool(name="const", bufs=1) as const,
        ):
            ag1_in = dram.tile([R, F1], f32)
            ag1_out = dram.tile([N, F1], f32, addr_space="Shared")
            ag2_in = dram.tile([R, F2], f32)
            ag2_out = dram.tile([N, F2], f32, addr_space="Shared")

            # constants (single DMAs with 3D APs to keep consumer wait
            # counts low -- walrus limits sync-waits per instruction)
            w1_sb = const.tile([P, K0 * F1], f32)
            nc.sync.dma_start(
                out=w1_sb[:].rearrange("p (k f) -> p k f", k=K0),
                in_=W1_d.rearrange("(k p) f -> p k f", p=P))
            w2_sb = const.tile([P, K2 * F2], f32)
            nc.sync.dma_start(
                out=w2_sb[:].rearrange("p (k f) -> p k f", k=K2),
                in_=W2_d.rearrange("(k p) f -> p k f", p=P))
            b1_row = const.tile([1, F1], f32)
            nc.sync.dma_start(out=b1_row[:, :], in_=b1_d[None, :])
            b2_row = const.tile([1, F2], f32)
            nc.sync.dma_start(out=b2_row[:, :], in_=b2_d[None, :])
            ones_col = const.tile([1, P], f32)
            nc.vector.memset(ones_col[:], 1.0)

            iota_i = const.tile([P, P], i32)
            nc.gpsimd.iota(iota_i[:], pattern=[[1, P]], base=0,
                           channel_multiplier=0)
            iota_f = const.tile([P, P], f32)
            nc.vector.tensor_copy(out=iota_f[:], in_=iota_i[:])
            ident = const.tile([P, P], f32)
            make_identity(nc, ident[:])

            srcs_sb = const.tile([P, total], i32)
            nc.sync.dma_start(out=srcs_sb[:], in_=srcs_d[:])
            dstl_sb = const.tile([P, total], f32)
            nc.sync.dma_start(out=dstl_sb[:], in_=dstl_d[:])
            nrm_sb = const.tile([P, total], f32)
            nc.sync.dma_start(out=nrm_sb[:], in_=nrm_d[:])

            h1T = const.tile([P, H1 * RP], f32)  # h1 transposed, H1 row-blocks

            # ---------------- phase 1: xw1 = x_c @ W1 ----------------
            with (
                tc.tile_pool(name="p1x", bufs=1) as p1x,
                tc.tile_pool(name="p1o", bufs=3) as p1o,
                tc.tile_pool(name="p1ps", bufs=2, space="PSUM") as p1ps,
            ):
                xt_sb = p1x.tile([P, K0 * R], f32)
                nc.sync.dma_start(
                    out=xt_sb[:].rearrange("p (k r) -> p k r", k=K0),
                    in_=xT.rearrange("(k p) r -> p k r", p=P))
                for m in range(ntiles):
                    rows = last_rows if m == ntiles - 1 else P
                    ps = p1ps.tile([P, F1], f32)
                    for k in range(K0):
                        nc.tensor.matmul(
                            out=ps[:rows, :],
                            lhsT=xt_sb[:, k * R + m * P: k * R + m * P + rows],
                            rhs=w1_sb[:, k * F1:(k + 1) * F1],
                            start=(k == 0), stop=(k == K0 - 1))
                    os = p1o.tile([P, F1], f32)
                    nc.scalar.activation(out=os[:rows, :], in_=ps[:rows, :],
                                         func=mybir.ActivationFunctionType.Copy)
                    nc.sync.dma_start(out=ag1_in[m * P: m * P + rows, :],
                                      in_=os[:rows, :])

            nc.gpsimd.collective_compute(
                "AllGather", mybir.AluOpType.bypass, replica_groups=rg,
                ins=[ag1_in[:].opt()], outs=[ag1_out[:].opt()])

            # ------- phase 2: aggregate layer 1 (node-major), then
            #         relu+bias and PE-transpose into h1T -------
            with (
                tc.tile_pool(name="p2g", bufs=8) as p2g,
                tc.tile_pool(name="p2s", bufs=4) as p2s,
                tc.tile_pool(name="p2h", bufs=3) as p2h,
                tc.tile_pool(name="p2ps", bufs=3, space="PSUM") as p2ps,
                tc.tile_pool(name="p2pt", bufs=3, space="PSUM") as p2pt,
            ):
                for t in range(ntiles):
                    nchunk = chunks[t]
                    ps = p2ps.tile([P, F1], f32, tag="ps", name=f"ps_{t}")
                    for c in range(nchunk):
                        j = offs[t] + c
                        G = p2g.tile([P, F1], f32, tag="G", name=f"G_{j}")
                        nc.gpsimd.indirect_dma_start(
                            out=G[:], out_offset=None, in_=ag1_out[:],
                            in_offset=bass.IndirectOffsetOnAxis(
                                ap=srcs_sb[:, j:j + 1], axis=0))
                        S = p2s.tile([P, P], f32, tag="S")
                        nc.vector.tensor_scalar(
                            out=S[:], in0=iota_f[:],
                            scalar1=dstl_sb[:, j:j + 1],
                            scalar2=nrm_sb[:, j:j + 1],
                            op0=mybir.AluOpType.is_equal,
                            op1=mybir.AluOpType.mult)
                        nc.tensor.matmul(
                            out=ps[:], lhsT=S[:], rhs=G[:],
                            start=(c == 0), stop=False)
                    # += ones^T @ b1 (adds b1 to every row)
                    nc.tensor.matmul(out=ps[:], lhsT=ones_col[:],
                                     rhs=b1_row[:], start=False, stop=True)
                    hm = p2h.tile([P, F1], f32, tag="hm")
                    nc.scalar.activation(
                        out=hm[:], in_=ps[:],
                        func=mybir.ActivationFunctionType.Relu)
                    for h in range(H1):
                        pt = p2pt.tile([P, P], f32, tag="pt")
                        nc.tensor.transpose(
                            out=pt[:], in_=hm[:, h * P:(h + 1) * P],
                            identity=ident[:])
                        nc.vector.tensor_copy(
                            out=h1T[:, h * RP + t * P: h * RP + (t + 1) * P],
                            in_=pt[:])

            # ---------------- phase 3: hw2 = h1 @ W2 ----------------
            with (
                tc.tile_pool(name="p3o", bufs=3) as p3o,
                tc.tile_pool(name="p3ps", bufs=2, space="PSUM") as p3ps,
            ):
                for m in range(ntiles):
                    rows = last_rows if m == ntiles - 1 else P
                    ps = p3ps.tile([P, F2], f32)
                    for k in range(K2):
                        nc.tensor.matmul(
                            out=ps[:rows, :],
                            lhsT=h1T[:, k * RP + m * P: k * RP + m * P + rows],
                            rhs=w2_sb[:, k * F2:(k + 1) * F2],
                            start=(k == 0), stop=(k == K2 - 1))
                    os = p3o.tile([P, F2], f32)
                    nc.scalar.activation(out=os[:rows, :], in_=ps[:rows, :],
                                         func=mybir.ActivationFunctionType.Copy)
                    nc.sync.dma_start(out=ag2_in[m * P: m * P + rows, :],
                                      in_=os[:rows, :])

            nc.gpsimd.collective_compute(
                "AllGather", mybir.AluOpType.bypass, replica_groups=rg,
                ins=[ag2_in[:].opt()], outs=[ag2_out[:].opt()])

            # ------- phase 4: aggregate layer 2, node-major out -------
            with (
                tc.tile_pool(name="p4g", bufs=6) as p4g,
                tc.tile_pool(name="p4s", bufs=4) as p4s,
                tc.tile_pool(name="p4o", bufs=3) as p4o,
                tc.tile_pool(name="p4ps", bufs=3, space="PSUM") as p4ps,
            ):
                for t in range(ntiles):
                    rows = last_rows if t == ntiles - 1 else P
                    ps = p4ps.tile([P, F2], f32)
                    for c in range(chunks[t]):
                        j = offs[t] + c
                        G2 = p4g.tile([P, F2], f32, tag="G2", name=f"G2_{j}")
                        nc.gpsimd.indirect_dma_start(
                            out=G2[:], out_offset=None, in_=ag2_out[:],
                            in_offset=bass.IndirectOffsetOnAxis(
                                ap=srcs_sb[:, j:j + 1], axis=0))
                        S = p4s.tile([P, P], f32, tag="S4")
                        nc.vector.tensor_scalar(
                            out=S[:], in0=iota_f[:],
                            scalar1=dstl_sb[:, j:j + 1],
                            scalar2=nrm_sb[:, j:j + 1],
                            op0=mybir.AluOpType.is_equal,
                            op1=mybir.AluOpType.mult)
                        nc.tensor.matmul(
                            out=ps[:], lhsT=S[:],
                            rhs=G2[:],
                            start=(c == 0), stop=False)
                    # bias: += ones^T @ b2  (rank-1, adds b2 to every row)
                    nc.tensor.matmul(out=ps[:], lhsT=ones_col[:],
                                     rhs=b2_row[:], start=False, stop=True)
                    os = p4o.tile([P, F2], f32)
                    nc.scalar.activation(out=os[:rows, :], in_=ps[:rows, :],
                                         func=mybir.ActivationFunctionType.Copy)
                    nc.sync.dma_start(out=out_d[t * P: t * P + rows, :],
                                      in_=os[:rows, :])

    nc.compile()
    return nc


# ----------------------------------------------------------------------------
# Public entry point
# ----------------------------------------------------------------------------

LAST_EXEC_NS = None
LAST_RESULTS = None


def kernel(x, edge_index, W1, b1, W2, b2, _trace=False, _tmpdir=None):
    global LAST_EXEC_NS, LAST_RESULTS
    x = np.asarray(x, np.float32)
    edge_index = np.asarray(edge_index)
    W1 = np.asarray(W1, np.float32)
    b1 = np.asarray(b1, np.float32)
    W2 = np.asarray(W2, np.float32)
    b2 = np.asarray(b2, np.float32)
    N, F0 = x.shape
    F1 = W1.shape[1]
    F2 = W2.shape[1]

    packed, chunks, R, ntiles = _preprocess(x, edge_index, NCORES)
    nc = build_nc(N, R, ntiles, chunks, F0, F1, F2, NCORES)

    in_maps = []
    for c in range(NCORES):
        s_a, d_a, n_a = packed[c]
        xT_c = np.ascontiguousarray(x[c * R:(c + 1) * R].T)
        in_maps.append({
            "xT": xT_c, "srcs": s_a, "dstl": d_a, "nrm": n_a,
            "W1": W1, "b1": b1, "W2": W2, "b2": b2,
        })

    res = bass_utils.run_bass_kernel_spmd(
        nc, in_maps, core_ids=list(range(NCORES)), trace=_trace,
        tmpdir=_tmpdir)
    LAST_EXEC_NS = res.exec_time_ns
    LAST_RESULTS = res
    out = np.concatenate([res.results[c]["out"] for c in range(NCORES)], axis=0)
    return out.astype(np.float32)



# revision 6
# speedup vs baseline: 1.1497x; 1.1497x over previous
"""2-layer GCN (PyG GCNConv style) on 8 Trainium2 NeuronCores.

Strategy (graph/node parallel, per sharding hint), v3:
  - Nodes range-sharded across 8 cores (R = N/8 rows each).
  - All matmul operands bf16 (PE 1 cycle/row vs 4 for fp32), fp32 PSUM.
  - Norm factoring: out = dis[dst] * sum_e (dis[src]*xw[src]).  Stored rows
    pre-scaled by dis[src] (ACT-engine scale on the phase-1/3 epilogue),
    output tiles post-scaled by dis[dst] (ACT epilogue).  The per-chunk
    selection matrix S is then pure 0/1 one-hot, built with ONE broadcast
    is_equal per gather batch on DVE.
  - Source-row gathers use gpsimd.dma_gather, batched over GB dst-tiles per
    instruction to amortize the ~1us fixed SWDGE cost.  dma_gather indices
    are int16, so the gather table is split in two halves (row < ha and
    row >= ha) and every (tile, half) gets its own chunks; two gather
    instructions per batch.  Slot i of an instruction lands in
    out[i%128, i//128, :], with idx value at [i%16, 8*chunk + (i%128)//16].
  - Device per core:
      phase 1: xw1 = x_c @ W1, rows scaled by dis  -> AllGather (bf16)
      phase 2+3 fused per dst tile: batched gathers, S one-hot,
               PE-matmul-accumulate S^T @ G, ReLU(dis * ps), PE-transpose
               to h1T, immediately h1 @ W2 (scaled by dis) -> ag2_in
      AllGather (bf16)
      phase 4: same aggregation, out = dis * (S^T @ G2), fp32 out.
  - Host concatenates the 8 row-shards.
"""

import sys

for p in ("/opt/trn_rl_repo",):
    if p not in sys.path:
        sys.path.insert(0, p)

import numpy as np
import ml_dtypes

import concourse.bass as bass
import concourse.bacc as bacc
import concourse.mybir as mybir
import concourse.tile as tile
from concourse import bass_utils
from concourse.masks import make_identity

P = 128
NCORES = 8
GB = 3            # dst tiles per gather batch (shared by both layers)
BF16 = ml_dtypes.bfloat16

PAD_DSTL = 255.0  # is_equal(255, d) is false for every d in 0..127


# ----------------------------------------------------------------------------
# Host-side preprocessing
# ----------------------------------------------------------------------------

def _preprocess(x, edge_index, n_cores, ha=None):
    """Pack per-core edge metadata for the batched dma_gather scheme."""
    N = x.shape[0]
    R = N // n_cores
    assert R * n_cores == N
    ntiles = (R + P - 1) // P

    src = edge_index[0].astype(np.int64)
    dst = edge_index[1].astype(np.int64)
    loops = np.arange(N, dtype=np.int64)
    src = np.concatenate([src, loops])
    dst = np.concatenate([dst, loops])

    deg = np.bincount(dst, minlength=N).astype(np.float64)
    dis = (1.0 / np.sqrt(deg)).astype(np.float32)  # deg>=1 via self-loops

    if ha is None:
        ha = N if N <= 32768 else (N + 1) // 2
    assert ha <= 32768 and (N - ha) <= 32768  # int16 gather indices

    core_id = dst // R
    dloc = dst - core_id * R
    tl = dloc // P
    dstl = (dloc - tl * P).astype(np.float32)
    half = (src >= ha).astype(np.int64)

    key = (core_id * ntiles + tl) * 2 + half
    counts = np.bincount(key, minlength=n_cores * ntiles * 2) \
        .reshape(n_cores, ntiles, 2)
    cmax = counts.max(axis=0)  # [ntiles, 2]
    cA = np.ceil(cmax[:, 0] / P).astype(np.int64)
    cB = np.ceil(cmax[:, 1] / P).astype(np.int64)

    # batch structure: per batch, A-chunks (grouped per tile) then B-chunks
    startA = np.zeros(ntiles, np.int64)
    startB = np.zeros(ntiles, np.int64)
    batches = []
    col = 0
    for t0 in range(0, ntiles, GB):
        t1 = min(t0 + GB, ntiles)
        j0 = col
        for t in range(t0, t1):
            startA[t] = col
            col += cA[t]
        kA = col - j0
        for t in range(t0, t1):
            startB[t] = col
            col += cB[t]
        kB = col - j0 - kA
        batches.append(dict(t0=t0, t1=t1, j0=int(j0), kA=int(kA), kB=int(kB)))
    total = int(col)

    tile_cols = [
        list(range(startA[t], startA[t] + cA[t]))
        + list(range(startB[t], startB[t] + cB[t]))
        for t in range(ntiles)
    ]
    layout = dict(batches=batches, tile_cols=tile_cols, total=total)

    RP = ntiles * P
    packed = []
    for c in range(n_cores):
        m = core_id == c
        s_c = src[m]
        t_c = tl[m]
        d_c = dstl[m]
        h_c = half[m]
        grp = t_c * 2 + h_c
        order = np.argsort(grp, kind="stable")
        s_c, t_c, d_c, h_c, grp = (a[order] for a in (s_c, t_c, d_c, h_c, grp))
        cnt = np.bincount(grp, minlength=ntiles * 2)
        starts = np.cumsum(cnt) - cnt
        pos = np.arange(len(grp)) - np.repeat(starts, cnt)
        start_col = np.where(h_c == 1, startB[t_c], startA[t_c])
        chunk_col = start_col + pos // P
        part = pos % P

        A_dl = np.full((P, total), PAD_DSTL, np.float32)
        A_dl[part, chunk_col] = d_c
        vals = np.where(h_c == 1, s_c - ha, s_c).astype(np.int16)
        idx16 = np.zeros((P, total * 8), np.int16)
        idx16[part % 16, chunk_col * 8 + part // 16] = vals
        # idx block must be replicated across the 8 Q7-core partition stripes
        idx16[16:, :] = np.tile(idx16[0:16, :], (7, 1))

        dis_c = np.zeros(RP, np.float32)
        dis_c[:R] = dis[c * R:(c + 1) * R]
        diso = np.ascontiguousarray(dis_c.reshape(ntiles, P).T)  # [P, ntiles]

        packed.append((idx16, A_dl.astype(BF16), diso))
    return packed, layout, R, ntiles, dis, ha


# ----------------------------------------------------------------------------
# Device kernel builder (parameterized so a tiny config can be sim-tested)
# ----------------------------------------------------------------------------

def build_nc(N, R, ntiles, layout, F0, F1, F2, n_cores, ha,
             has_b1=False, has_b2=False):
    """Build the SPMD Bass program. F0,F1,F2 multiples of 128."""
    f32 = mybir.dt.float32
    bf = mybir.dt.bfloat16
    i16 = mybir.dt.int16
    AF = mybir.ActivationFunctionType
    K0 = F0 // P       # k-tiles in layer-1 matmul
    H1 = F1 // P       # 128-wide blocks of F1
    K2 = F1 // P       # k-tiles in layer-2 matmul (= H1)
    assert F2 <= 512 and F2 % P == 0
    last_rows = R - (ntiles - 1) * P
    RP = ntiles * P    # padded row count
    total = layout["total"]
    batches = layout["batches"]
    tile_cols = layout["tile_cols"]

    nc = bacc.Bacc("TRN2", target_bir_lowering=False, debug=False,
                   num_devices=n_cores)

    xT = nc.dram_tensor("xT", [F0, R], bf, kind="ExternalInput").ap()
    idx_d = nc.dram_tensor("idx16", [P, total * 8], i16,
                           kind="ExternalInput").ap()
    dstl_d = nc.dram_tensor("dstl", [P, total], bf, kind="ExternalInput").ap()
    diso_d = nc.dram_tensor("diso", [P, ntiles], f32, kind="ExternalInput").ap()
    W1_d = nc.dram_tensor("W1", [F0, F1], bf, kind="ExternalInput").ap()
    W2_d = nc.dram_tensor("W2", [F1, F2], bf, kind="ExternalInput").ap()
    if has_b1:
        b1_d = nc.dram_tensor("b1", [F1], bf, kind="ExternalInput").ap()
    if has_b2:
        b2_d = nc.dram_tensor("b2", [F2], bf, kind="ExternalInput").ap()
    if has_b1 or has_b2:
        invd_d = nc.dram_tensor("invd", [1, RP], bf, kind="ExternalInput").ap()
    out_d = nc.dram_tensor("out", [R, F2], f32, kind="ExternalOutput").ap()

    rg = [list(range(n_cores))]

    with tile.TileContext(nc) as tc:
        with (
            tc.tile_pool(name="dram", bufs=1, space="DRAM") as dram,
            tc.tile_pool(name="const", bufs=1) as const,
        ):
            # NOTE: Local (not Shared) outputs — the dma_gather ucode reads
            # garbage / faults when the table lives in the Shared scratchpad.
            ag1_in = dram.tile([R, F1], bf)
            ag1_out = dram.tile([N, F1], bf)
            ag2_in = dram.tile([R, F2], bf)
            ag2_out = dram.tile([N, F2], bf)

            w1_sb = const.tile([P, K0 * F1], bf)
            nc.sync.dma_start(
                out=w1_sb[:].rearrange("p (k f) -> p k f", k=K0),
                in_=W1_d.rearrange("(k p) f -> p k f", p=P))
            w2_sb = const.tile([P, K2 * F2], bf)
            nc.sync.dma_start(
                out=w2_sb[:].rearrange("p (k f) -> p k f", k=K2),
                in_=W2_d.rearrange("(k p) f -> p k f", p=P))
            if has_b1:
                b1_row = const.tile([1, F1], bf)
                nc.sync.dma_start(out=b1_row[:, :], in_=b1_d[None, :])
            if has_b2:
                b2_row = const.tile([1, F2], bf)
                nc.sync.dma_start(out=b2_row[:, :], in_=b2_d[None, :])
            if has_b1 or has_b2:
                invd_sb = const.tile([1, RP], bf)
                nc.sync.dma_start(out=invd_sb[:, :], in_=invd_d[:, :])

            iota_i = const.tile([P, P], mybir.dt.int32)
            nc.gpsimd.iota(iota_i[:], pattern=[[1, P]], base=0,
                           channel_multiplier=0)
            iota_bf = const.tile([P, P], bf)
            nc.vector.tensor_copy(out=iota_bf[:], in_=iota_i[:])
            ident = const.tile([P, P], bf)
            make_identity(nc, ident[:])

            idx_sb = const.tile([P, total * 8], i16)
            nc.sync.dma_start(out=idx_sb[:], in_=idx_d[:])
            dstl_sb = const.tile([P, total], bf)
            nc.sync.dma_start(out=dstl_sb[:], in_=dstl_d[:])
            diso_sb = const.tile([P, ntiles], f32)
            nc.sync.dma_start(out=diso_sb[:], in_=diso_d[:])

            h1T = const.tile([P, H1 * RP], bf)  # h1 transposed, H1 row-blocks

            def build_S(pool, j0, kb, tag):
                """One-hot S[e, c*128+d] = (dstl[e, j0+c] == d) per batch."""
                S = pool.tile([P, kb * P], bf, tag=tag)
                nc.vector.scalar_tensor_tensor(
                    out=S[:].rearrange("p (k d) -> p k d", k=kb),
                    in0=dstl_sb[:, j0:j0 + kb].unsqueeze(2)
                        .broadcast_to([P, kb, P]),
                    scalar=1.0,
                    in1=iota_bf[:].unsqueeze(1).broadcast_to([P, kb, P]),
                    op0=mybir.AluOpType.mult,
                    op1=mybir.AluOpType.is_equal)
                return S

            GMAX = 8  # max chunks (x128 idxs) per dma_gather instruction

            def gather_batch(pool, b, src_full, F, tag):
                """dma_gathers (per table half, split at GMAX) for one batch."""
                kb = b["kA"] + b["kB"]
                G = pool.tile([P, kb * F], bf, tag=tag,
                              name=f"{tag}_{b['t0']}")
                for (k, coff, lo, hi) in (
                    (b["kA"], 0, 0, ha),
                    (b["kB"], b["kA"], ha, N),
                ):
                    for c0 in range(0, k, GMAX):
                        kk = min(GMAX, k - c0)
                        j = b["j0"] + coff + c0
                        o = coff + c0
                        nc.gpsimd.dma_gather(
                            out_ap=G[:, o * F:(o + kk) * F]
                                .rearrange("p (k f) -> p k f", k=kk),
                            in_ap=src_full[lo:hi, :],
                            idxs_ap=idx_sb[:, 8 * j: 8 * (j + kk)],
                            num_idxs=kk * P,
                            num_idxs_reg=kk * P,
                            elem_size=F)
                return G

            # ---------------- phase 1: xw1 = dis * (x_c @ W1) ----------------
            with (
                tc.tile_pool(name="p1x", bufs=1) as p1x,
                tc.tile_pool(name="p1o", bufs=3) as p1o,
                tc.tile_pool(name="p1ps", bufs=2, space="PSUM") as p1ps,
            ):
                xt_sb = p1x.tile([P, K0 * R], bf)
                nc.sync.dma_start(
                    out=xt_sb[:].rearrange("p (k r) -> p k r", k=K0),
                    in_=xT.rearrange("(k p) r -> p k r", p=P))
                for m in range(ntiles):
                    rows = last_rows if m == ntiles - 1 else P
                    ps = p1ps.tile([P, F1], f32)
                    for k in range(K0):
                        nc.tensor.matmul(
                            out=ps[:rows, :],
                            lhsT=xt_sb[:, k * R + m * P: k * R + m * P + rows],
                            rhs=w1_sb[:, k * F1:(k + 1) * F1],
                            start=(k == 0), stop=(k == K0 - 1))
                    os = p1o.tile([P, F1], bf)
                    nc.scalar.activation(out=os[:rows, :], in_=ps[:rows, :],
                                         func=AF.Copy,
                                         scale=diso_sb[:rows, m:m + 1])
                    nc.sync.dma_start(out=ag1_in[m * P: m * P + rows, :],
                                      in_=os[:rows, :])

            nc.gpsimd.collective_compute(
                "AllGather", mybir.AluOpType.bypass, replica_groups=rg,
                ins=[ag1_in[:].opt()], outs=[ag1_out[:].opt()])

            # ------- phase 2 (+3 fused): aggregate layer 1, relu, h1T,
            #         hw2 = h1 @ W2, write ag2_in -------
            with (
                tc.tile_pool(name="p2g", bufs=3) as p2g,
                tc.tile_pool(name="p2s", bufs=3) as p2s,
                tc.tile_pool(name="p2h", bufs=3) as p2h,
                tc.tile_pool(name="p2o", bufs=3) as p2o,
                tc.tile_pool(name="p2ps", bufs=2, space="PSUM") as p2ps,
                tc.tile_pool(name="p2pt", bufs=2, space="PSUM") as p2pt,
                tc.tile_pool(name="p2p3", bufs=2, space="PSUM") as p2p3,
            ):
                for b in batches:
                    j0, kb = b["j0"], b["kA"] + b["kB"]
                    G = gather_batch(p2g, b, ag1_out, F1, "G")
                    S = build_S(p2s, j0, kb, "S")
                    for t in range(b["t0"], b["t1"]):
                        rows = last_rows if t == ntiles - 1 else P
                        cols = tile_cols[t]
                        ps = p2ps.tile([P, F1], f32, tag="ps")
                        for i, ccol in enumerate(cols):
                            rc = ccol - j0
                            nc.tensor.matmul(
                                out=ps[:],
                                lhsT=S[:, rc * P:(rc + 1) * P],
                                rhs=G[:, rc * F1:(rc + 1) * F1],
                                start=(i == 0),
                                stop=(i == len(cols) - 1 and not has_b1))
                        if has_b1:
                            nc.tensor.matmul(
                                out=ps[:], lhsT=invd_sb[:, t * P:(t + 1) * P],
                                rhs=b1_row[:], start=False, stop=True)
                        hm = p2h.tile([P, F1], bf, tag="hm")
                        nc.scalar.activation(out=hm[:], in_=ps[:],
                                             func=AF.Relu,
                                             scale=diso_sb[:, t:t + 1])
                        for h in range(H1):
                            pt = p2pt.tile([P, P], bf, tag="pt")
                            nc.tensor.transpose(
                                out=pt[:], in_=hm[:, h * P:(h + 1) * P],
                                identity=ident[:])
                            nc.scalar.activation(
                                out=h1T[:, h * RP + t * P: h * RP + (t + 1) * P],
                                in_=pt[:], func=AF.Copy)
                        p3 = p2p3.tile([P, F2], f32, tag="p3")
                        for k in range(K2):
                            nc.tensor.matmul(
                                out=p3[:rows, :],
                                lhsT=h1T[:, k * RP + t * P: k * RP + t * P + rows],
                                rhs=w2_sb[:, k * F2:(k + 1) * F2],
                                start=(k == 0), stop=(k == K2 - 1))
                        o3 = p2o.tile([P, F2], bf, tag="o3")
                        nc.scalar.activation(out=o3[:rows, :], in_=p3[:rows, :],
                                             func=AF.Copy,
                                             scale=diso_sb[:rows, t:t + 1])
                        nc.sync.dma_start(out=ag2_in[t * P: t * P + rows, :],
                                          in_=o3[:rows, :])

            nc.gpsimd.collective_compute(
                "AllGather", mybir.AluOpType.bypass, replica_groups=rg,
                ins=[ag2_in[:].opt()], outs=[ag2_out[:].opt()])

            # ------- phase 4: aggregate layer 2, node-major fp32 out -------
            with (
                tc.tile_pool(name="p4g", bufs=3) as p4g,
                tc.tile_pool(name="p4s", bufs=3) as p4s,
                tc.tile_pool(name="p4o", bufs=3) as p4o,
                tc.tile_pool(name="p4ps", bufs=3, space="PSUM") as p4ps,
            ):
                for b in batches:
                    j0, kb = b["j0"], b["kA"] + b["kB"]
                    G2 = gather_batch(p4g, b, ag2_out, F2, "G2")
                    S = build_S(p4s, j0, kb, "S4")
                    for t in range(b["t0"], b["t1"]):
                        rows = last_rows if t == ntiles - 1 else P
                        cols = tile_cols[t]
                        ps = p4ps.tile([P, F2], f32, tag="ps4")
                        for i, ccol in enumerate(cols):
                            rc = ccol - j0
                            nc.tensor.matmul(
                                out=ps[:],
                                lhsT=S[:, rc * P:(rc + 1) * P],
                                rhs=G2[:, rc * F2:(rc + 1) * F2],
                                start=(i == 0),
                                stop=(i == len(cols) - 1 and not has_b2))
                        if has_b2:
                            nc.tensor.matmul(
                                out=ps[:], lhsT=invd_sb[:, t * P:(t + 1) * P],
                                rhs=b2_row[:], start=False, stop=True)
                        ot = p4o.tile([P, F2], f32, tag="ot")
                        nc.scalar.activation(out=ot[:rows, :], in_=ps[:rows, :],
                                             func=AF.Copy,
                                             scale=diso_sb[:rows, t:t + 1])
                        nc.sync.dma_start(out=out_d[t * P: t * P + rows, :],
                                          in_=ot[:rows, :])

    nc.compile()
    return nc


# ----------------------------------------------------------------------------
# Public entry point
# ----------------------------------------------------------------------------

LAST_EXEC_NS = None
LAST_RESULTS = None


def kernel(x, edge_index, W1, b1, W2, b2, _trace=False, _tmpdir=None):
    global LAST_EXEC_NS, LAST_RESULTS
    x = np.asarray(x, np.float32)
    edge_index = np.asarray(edge_index)
    W1 = np.asarray(W1, np.float32)
    b1 = np.asarray(b1, np.float32)
    W2 = np.asarray(W2, np.float32)
    b2 = np.asarray(b2, np.float32)
    N, F0 = x.shape
    F1 = W1.shape[1]
    F2 = W2.shape[1]
    has_b1 = bool(np.any(b1 != 0))
    has_b2 = bool(np.any(b2 != 0))

    packed, layout, R, ntiles, dis, ha = _preprocess(x, edge_index, NCORES)
    nc = build_nc(N, R, ntiles, layout, F0, F1, F2, NCORES, ha,
                  has_b1=has_b1, has_b2=has_b2)

    RP = ntiles * P
    in_maps = []
    for c in range(NCORES):
        idx16, d_a, diso = packed[c]
        xT_c = np.ascontiguousarray(x[c * R:(c + 1) * R].T).astype(BF16)
        im = {
            "xT": xT_c, "idx16": idx16, "dstl": d_a, "diso": diso,
            "W1": W1.astype(BF16), "W2": W2.astype(BF16),
        }
        if has_b1:
            im["b1"] = b1.astype(BF16)
        if has_b2:
            im["b2"] = b2.astype(BF16)
        if has_b1 or has_b2:
            invd = np.zeros((1, RP), np.float32)
            invd[0, :R] = 1.0 / dis[c * R:(c + 1) * R]
            im["invd"] = invd.astype(BF16)
        in_maps.append(im)

    res = bass_utils.run_bass_kernel_spmd(
        nc, in_maps, core_ids=list(range(NCORES)), trace=_trace,
        tmpdir=_tmpdir)
    LAST_EXEC_NS = res.exec_time_ns
    LAST_RESULTS = res
    out = np.concatenate([res.results[c]["out"] for c in range(NCORES)], axis=0)
    return out.astype(np.float32)


# revision 8
# speedup vs baseline: 2.2617x; 1.9673x over previous
"""2-layer GCN (PyG GCNConv style) on 8 Trainium2 NeuronCores.

Strategy (graph/node parallel, per sharding hint), v3:
  - Nodes range-sharded across 8 cores (R = N/8 rows each).
  - All matmul operands bf16 (PE 1 cycle/row vs 4 for fp32), fp32 PSUM.
  - Norm factoring: out = dis[dst] * sum_e (dis[src]*xw[src]).  Stored rows
    pre-scaled by dis[src] (ACT-engine scale on the phase-1/3 epilogue),
    output tiles post-scaled by dis[dst] (ACT epilogue).  The per-chunk
    selection matrix S is then pure 0/1 one-hot, built with ONE broadcast
    is_equal per gather batch on DVE.
  - Source-row gathers use gpsimd.dma_gather, batched over GB dst-tiles per
    instruction to amortize the ~1us fixed SWDGE cost.  dma_gather indices
    are int16, so the gather table is split in two halves (row < ha and
    row >= ha) and every (tile, half) gets its own chunks; two gather
    instructions per batch.  Slot i of an instruction lands in
    out[i%128, i//128, :], with idx value at [i%16, 8*chunk + (i%128)//16].
  - Device per core:
      phase 1: xw1 = x_c @ W1, rows scaled by dis  -> AllGather (bf16)
      phase 2+3 fused per dst tile: batched gathers, S one-hot,
               PE-matmul-accumulate S^T @ G, ReLU(dis * ps), PE-transpose
               to h1T, immediately h1 @ W2 (scaled by dis) -> ag2_in
      AllGather (bf16)
      phase 4: same aggregation, out = dis * (S^T @ G2), fp32 out.
  - Host concatenates the 8 row-shards.
"""

import sys

for p in ("/opt/trn_rl_repo",):
    if p not in sys.path:
        sys.path.insert(0, p)

import numpy as np
import ml_dtypes

import concourse.bass as bass
import concourse.bacc as bacc
import concourse.mybir as mybir
import concourse.tile as tile
from concourse import bass_utils
from concourse.masks import make_identity

P = 128
NCORES = 8
GB = 3            # dst tiles per gather batch (shared by both layers)
BF16 = ml_dtypes.bfloat16

PAD_DSTL = 255.0  # is_equal(255, d) is false for every d in 0..127


# ----------------------------------------------------------------------------
# Host-side preprocessing
# ----------------------------------------------------------------------------

def _preprocess(x, edge_index, n_cores, ha=None):
    """Pack per-core edge metadata for the batched dma_gather scheme."""
    N = x.shape[0]
    R = N // n_cores
    assert R * n_cores == N
    ntiles = (R + P - 1) // P

    src = edge_index[0].astype(np.int64)
    dst = edge_index[1].astype(np.int64)
    loops = np.arange(N, dtype=np.int64)
    src = np.concatenate([src, loops])
    dst = np.concatenate([dst, loops])

    deg = np.bincount(dst, minlength=N).astype(np.float64)
    dis = (1.0 / np.sqrt(deg)).astype(np.float32)  # deg>=1 via self-loops

    if ha is None:
        ha = N if N <= 32768 else (N + 1) // 2
    assert ha <= 32768 and (N - ha) <= 32768  # int16 gather indices

    core_id = dst // R
    dloc = dst - core_id * R
    tl = dloc // P
    dstl = (dloc - tl * P).astype(np.float32)
    half = (src >= ha).astype(np.int64)

    key = (core_id * ntiles + tl) * 2 + half
    counts = np.bincount(key, minlength=n_cores * ntiles * 2) \
        .reshape(n_cores, ntiles, 2)
    cmax = counts.max(axis=0)  # [ntiles, 2]
    cA = np.ceil(cmax[:, 0] / P).astype(np.int64)
    cB = np.ceil(cmax[:, 1] / P).astype(np.int64)

    # batch structure: per batch, A-chunks (grouped per tile) then B-chunks
    startA = np.zeros(ntiles, np.int64)
    startB = np.zeros(ntiles, np.int64)
    batches = []
    col = 0
    for t0 in range(0, ntiles, GB):
        t1 = min(t0 + GB, ntiles)
        j0 = col
        for t in range(t0, t1):
            startA[t] = col
            col += cA[t]
        kA = col - j0
        for t in range(t0, t1):
            startB[t] = col
            col += cB[t]
        kB = col - j0 - kA
        batches.append(dict(t0=t0, t1=t1, j0=int(j0), kA=int(kA), kB=int(kB)))
    total = int(col)

    tile_cols = [
        list(range(startA[t], startA[t] + cA[t]))
        + list(range(startB[t], startB[t] + cB[t]))
        for t in range(ntiles)
    ]
    layout = dict(batches=batches, tile_cols=tile_cols, total=total)

    RP = ntiles * P
    packed = []
    for c in range(n_cores):
        m = core_id == c
        s_c = src[m]
        t_c = tl[m]
        d_c = dstl[m]
        h_c = half[m]
        grp = t_c * 2 + h_c
        order = np.argsort(grp, kind="stable")
        s_c, t_c, d_c, h_c, grp = (a[order] for a in (s_c, t_c, d_c, h_c, grp))
        cnt = np.bincount(grp, minlength=ntiles * 2)
        starts = np.cumsum(cnt) - cnt
        pos = np.arange(len(grp)) - np.repeat(starts, cnt)
        start_col = np.where(h_c == 1, startB[t_c], startA[t_c])
        chunk_col = start_col + pos // P
        part = pos % P

        A_dl = np.full((P, total), PAD_DSTL, np.float32)
        A_dl[part, chunk_col] = d_c
        vals = np.where(h_c == 1, s_c - ha, s_c).astype(np.int16)
        idx16 = np.zeros((P, total * 8), np.int16)
        idx16[part % 16, chunk_col * 8 + part // 16] = vals
        # idx block must be replicated across the 8 Q7-core partition stripes
        idx16[16:, :] = np.tile(idx16[0:16, :], (7, 1))

        dis_c = np.zeros(RP, np.float32)
        dis_c[:R] = dis[c * R:(c + 1) * R]
        diso = np.ascontiguousarray(dis_c.reshape(ntiles, P).T)  # [P, ntiles]

        packed.append((idx16, A_dl.astype(BF16), diso))
    return packed, layout, R, ntiles, dis, ha


# ----------------------------------------------------------------------------
# Device kernel builder (parameterized so a tiny config can be sim-tested)
# ----------------------------------------------------------------------------

def build_nc(N, R, ntiles, layout, F0, F1, F2, n_cores, ha,
             has_b1=False, has_b2=False):
    """Build the SPMD Bass program. F0,F1,F2 multiples of 128."""
    f32 = mybir.dt.float32
    bf = mybir.dt.bfloat16
    i16 = mybir.dt.int16
    AF = mybir.ActivationFunctionType
    K0 = F0 // P       # k-tiles in layer-1 matmul
    H1 = F1 // P       # 128-wide blocks of F1
    K2 = F1 // P       # k-tiles in layer-2 matmul (= H1)
    assert F2 <= 512 and F2 % P == 0
    last_rows = R - (ntiles - 1) * P
    RP = ntiles * P    # padded row count
    total = layout["total"]
    batches = layout["batches"]
    tile_cols = layout["tile_cols"]

    nc = bacc.Bacc("TRN2", target_bir_lowering=False, debug=False,
                   num_devices=n_cores, num_swdge_queues=4)

    xT = nc.dram_tensor("xT", [F0, R], bf, kind="ExternalInput").ap()
    idx_d = nc.dram_tensor("idx16", [P, total * 8], i16,
                           kind="ExternalInput").ap()
    dstl_d = nc.dram_tensor("dstl", [P, total], bf, kind="ExternalInput").ap()
    diso_d = nc.dram_tensor("diso", [P, ntiles], f32, kind="ExternalInput").ap()
    W1_d = nc.dram_tensor("W1", [F0, F1], bf, kind="ExternalInput").ap()
    W2_d = nc.dram_tensor("W2", [F1, F2], bf, kind="ExternalInput").ap()
    if has_b1:
        b1_d = nc.dram_tensor("b1", [F1], bf, kind="ExternalInput").ap()
    if has_b2:
        b2_d = nc.dram_tensor("b2", [F2], bf, kind="ExternalInput").ap()
    if has_b1 or has_b2:
        invd_d = nc.dram_tensor("invd", [1, RP], bf, kind="ExternalInput").ap()
    out_d = nc.dram_tensor("out", [R, F2], f32, kind="ExternalOutput").ap()

    rg = [list(range(n_cores))]

    with tile.TileContext(nc) as tc:
        with (
            tc.tile_pool(name="dram", bufs=1, space="DRAM") as dram,
            tc.tile_pool(name="const", bufs=1) as const,
        ):
            # NOTE: Local (not Shared) outputs — the dma_gather ucode reads
            # garbage / faults when the table lives in the Shared scratchpad.
            ag1_in = dram.tile([R, F1], bf)
            ag1_out = dram.tile([N, F1], bf)
            ag2_in = dram.tile([R, F2], bf)
            ag2_out = dram.tile([N, F2], bf)

            w1_sb = const.tile([P, K0 * F1], bf)
            nc.sync.dma_start(
                out=w1_sb[:].rearrange("p (k f) -> p k f", k=K0),
                in_=W1_d.rearrange("(k p) f -> p k f", p=P))
            w2_sb = const.tile([P, K2 * F2], bf)
            nc.sync.dma_start(
                out=w2_sb[:].rearrange("p (k f) -> p k f", k=K2),
                in_=W2_d.rearrange("(k p) f -> p k f", p=P))
            if has_b1:
                b1_row = const.tile([1, F1], bf)
                nc.sync.dma_start(out=b1_row[:, :], in_=b1_d[None, :])
            if has_b2:
                b2_row = const.tile([1, F2], bf)
                nc.sync.dma_start(out=b2_row[:, :], in_=b2_d[None, :])
            if has_b1 or has_b2:
                invd_sb = const.tile([1, RP], bf)
                nc.sync.dma_start(out=invd_sb[:, :], in_=invd_d[:, :])

            iota_i = const.tile([P, P], mybir.dt.int32)
            nc.gpsimd.iota(iota_i[:], pattern=[[1, P]], base=0,
                           channel_multiplier=0)
            iota_bf = const.tile([P, P], bf)
            nc.vector.tensor_copy(out=iota_bf[:], in_=iota_i[:])
            ident = const.tile([P, P], bf)
            make_identity(nc, ident[:])

            idx_sb = const.tile([P, total * 8], i16)
            nc.sync.dma_start(out=idx_sb[:], in_=idx_d[:])
            dstl_sb = const.tile([P, total], bf)
            nc.sync.dma_start(out=dstl_sb[:], in_=dstl_d[:])
            diso_sb = const.tile([P, ntiles], f32)
            nc.sync.dma_start(out=diso_sb[:], in_=diso_d[:])

            h1T = const.tile([P, H1 * RP], bf)  # h1 transposed, H1 row-blocks

            def build_S(pool, j0, kb, tag):
                """One-hot S[e, c*128+d] = (dstl[e, j0+c] == d) per batch."""
                S = pool.tile([P, kb * P], bf, tag=tag)
                nc.vector.scalar_tensor_tensor(
                    out=S[:].rearrange("p (k d) -> p k d", k=kb),
                    in0=dstl_sb[:, j0:j0 + kb].unsqueeze(2)
                        .broadcast_to([P, kb, P]),
                    scalar=1.0,
                    in1=iota_bf[:].unsqueeze(1).broadcast_to([P, kb, P]),
                    op0=mybir.AluOpType.mult,
                    op1=mybir.AluOpType.is_equal)
                return S

            GMAX = 8  # max chunks (x128 idxs) per dma_gather instruction
            qrr = [0]  # SWDGE queue round-robin across gather instructions

            def gather_batch(pool, b, src_full, F, tag):
                """dma_gathers (per table half, split at GMAX) for one batch."""
                kb = b["kA"] + b["kB"]
                G = pool.tile([P, kb * F], bf, tag=tag,
                              name=f"{tag}_{b['t0']}")
                for (k, coff, lo, hi) in (
                    (b["kA"], 0, 0, ha),
                    (b["kB"], b["kA"], ha, N),
                ):
                    for c0 in range(0, k, GMAX):
                        kk = min(GMAX, k - c0)
                        j = b["j0"] + coff + c0
                        o = coff + c0
                        nc.gpsimd.dma_gather(
                            out_ap=G[:, o * F:(o + kk) * F]
                                .rearrange("p (k f) -> p k f", k=kk),
                            in_ap=src_full[lo:hi, :],
                            idxs_ap=idx_sb[:, 8 * j: 8 * (j + kk)],
                            num_idxs=kk * P,
                            num_idxs_reg=kk * P,
                            elem_size=F,
                            queue_num=qrr[0])
                        qrr[0] = (qrr[0] + 1) % 4
                return G

            # ---------------- phase 1: xw1 = dis * (x_c @ W1) ----------------
            with (
                tc.tile_pool(name="p1x", bufs=1) as p1x,
                tc.tile_pool(name="p1o", bufs=3) as p1o,
                tc.tile_pool(name="p1ps", bufs=2, space="PSUM") as p1ps,
            ):
                xt_sb = p1x.tile([P, K0 * R], bf)
                nc.sync.dma_start(
                    out=xt_sb[:].rearrange("p (k r) -> p k r", k=K0),
                    in_=xT.rearrange("(k p) r -> p k r", p=P))
                for m in range(ntiles):
                    rows = last_rows if m == ntiles - 1 else P
                    ps = p1ps.tile([P, F1], f32)
                    for k in range(K0):
                        nc.tensor.matmul(
                            out=ps[:rows, :],
                            lhsT=xt_sb[:, k * R + m * P: k * R + m * P + rows],
                            rhs=w1_sb[:, k * F1:(k + 1) * F1],
                            start=(k == 0), stop=(k == K0 - 1))
                    os = p1o.tile([P, F1], bf)
                    nc.scalar.activation(out=os[:rows, :], in_=ps[:rows, :],
                                         func=AF.Copy,
                                         scale=diso_sb[:rows, m:m + 1])
                    nc.sync.dma_start(out=ag1_in[m * P: m * P + rows, :],
                                      in_=os[:rows, :])

            nc.gpsimd.collective_compute(
                "AllGather", mybir.AluOpType.bypass, replica_groups=rg,
                ins=[ag1_in[:].opt()], outs=[ag1_out[:].opt()])

            # ------- phase 2 (+3 fused): aggregate layer 1, relu, h1T,
            #         hw2 = h1 @ W2, write ag2_in -------
            with (
                tc.tile_pool(name="p2g", bufs=3) as p2g,
                tc.tile_pool(name="p2s", bufs=3) as p2s,
                tc.tile_pool(name="p2h", bufs=3) as p2h,
                tc.tile_pool(name="p2o", bufs=3) as p2o,
                tc.tile_pool(name="p2ps", bufs=2, space="PSUM") as p2ps,
                tc.tile_pool(name="p2pt", bufs=2, space="PSUM") as p2pt,
                tc.tile_pool(name="p2p3", bufs=2, space="PSUM") as p2p3,
            ):
                for b in batches:
                    j0, kb = b["j0"], b["kA"] + b["kB"]
                    G = gather_batch(p2g, b, ag1_out, F1, "G")
                    S = build_S(p2s, j0, kb, "S")
                    for t in range(b["t0"], b["t1"]):
                        rows = last_rows if t == ntiles - 1 else P
                        cols = tile_cols[t]
                        ps = p2ps.tile([P, F1], f32, tag="ps")
                        for i, ccol in enumerate(cols):
                            rc = ccol - j0
                            nc.tensor.matmul(
                                out=ps[:],
                                lhsT=S[:, rc * P:(rc + 1) * P],
                                rhs=G[:, rc * F1:(rc + 1) * F1],
                                start=(i == 0),
                                stop=(i == len(cols) - 1 and not has_b1))
                        if has_b1:
                            nc.tensor.matmul(
                                out=ps[:], lhsT=invd_sb[:, t * P:(t + 1) * P],
                                rhs=b1_row[:], start=False, stop=True)
                        hm = p2h.tile([P, F1], bf, tag="hm")
                        nc.scalar.activation(out=hm[:], in_=ps[:],
                                             func=AF.Relu,
                                             scale=diso_sb[:, t:t + 1])
                        for h in range(H1):
                            pt = p2pt.tile([P, P], bf, tag="pt")
                            nc.tensor.transpose(
                                out=pt[:], in_=hm[:, h * P:(h + 1) * P],
                                identity=ident[:])
                            nc.scalar.activation(
                                out=h1T[:, h * RP + t * P: h * RP + (t + 1) * P],
                                in_=pt[:], func=AF.Copy)
                        p3 = p2p3.tile([P, F2], f32, tag="p3")
                        for k in range(K2):
                            nc.tensor.matmul(
                                out=p3[:rows, :],
                                lhsT=h1T[:, k * RP + t * P: k * RP + t * P + rows],
                                rhs=w2_sb[:, k * F2:(k + 1) * F2],
                                start=(k == 0), stop=(k == K2 - 1))
                        o3 = p2o.tile([P, F2], bf, tag="o3")
                        nc.scalar.activation(out=o3[:rows, :], in_=p3[:rows, :],
                                             func=AF.Copy,
                                             scale=diso_sb[:rows, t:t + 1])
                        nc.sync.dma_start(out=ag2_in[t * P: t * P + rows, :],
                                          in_=o3[:rows, :])

            nc.gpsimd.collective_compute(
                "AllGather", mybir.AluOpType.bypass, replica_groups=rg,
                ins=[ag2_in[:].opt()], outs=[ag2_out[:].opt()])

            # ------- phase 4: aggregate layer 2, node-major fp32 out -------
            with (
                tc.tile_pool(name="p4g", bufs=3) as p4g,
                tc.tile_pool(name="p4s", bufs=3) as p4s,
                tc.tile_pool(name="p4o", bufs=3) as p4o,
                tc.tile_pool(name="p4ps", bufs=3, space="PSUM") as p4ps,
            ):
                for b in batches:
                    j0, kb = b["j0"], b["kA"] + b["kB"]
                    G2 = gather_batch(p4g, b, ag2_out, F2, "G2")
                    S = build_S(p4s, j0, kb, "S4")
                    for t in range(b["t0"], b["t1"]):
                        rows = last_rows if t == ntiles - 1 else P
                        cols = tile_cols[t]
                        ps = p4ps.tile([P, F2], f32, tag="ps4")
                        for i, ccol in enumerate(cols):
                            rc = ccol - j0
                            nc.tensor.matmul(
                                out=ps[:],
                                lhsT=S[:, rc * P:(rc + 1) * P],
                                rhs=G2[:, rc * F2:(rc + 1) * F2],
                                start=(i == 0),
                                stop=(i == len(cols) - 1 and not has_b2))
                        if has_b2:
                            nc.tensor.matmul(
                                out=ps[:], lhsT=invd_sb[:, t * P:(t + 1) * P],
                                rhs=b2_row[:], start=False, stop=True)
                        ot = p4o.tile([P, F2], f32, tag="ot")
                        nc.scalar.activation(out=ot[:rows, :], in_=ps[:rows, :],
                                             func=AF.Copy,
                                             scale=diso_sb[:rows, t:t + 1])
                        nc.sync.dma_start(out=out_d[t * P: t * P + rows, :],
                                          in_=ot[:rows, :])

    nc.compile()
    return nc


# ----------------------------------------------------------------------------
# Public entry point
# ----------------------------------------------------------------------------

LAST_EXEC_NS = None
LAST_RESULTS = None


def kernel(x, edge_index, W1, b1, W2, b2, _trace=False, _tmpdir=None):
    global LAST_EXEC_NS, LAST_RESULTS
    x = np.asarray(x, np.float32)
    edge_index = np.asarray(edge_index)
    W1 = np.asarray(W1, np.float32)
    b1 = np.asarray(b1, np.float32)
    W2 = np.asarray(W2, np.float32)
    b2 = np.asarray(b2, np.float32)
    N, F0 = x.shape
    F1 = W1.shape[1]
    F2 = W2.shape[1]
    has_b1 = bool(np.any(b1 != 0))
    has_b2 = bool(np.any(b2 != 0))

    packed, layout, R, ntiles, dis, ha = _preprocess(x, edge_index, NCORES)
    nc = build_nc(N, R, ntiles, layout, F0, F1, F2, NCORES, ha,
                  has_b1=has_b1, has_b2=has_b2)

    RP = ntiles * P
    in_maps = []
    for c in range(NCORES):
        idx16, d_a, diso = packed[c]
        xT_c = np.ascontiguousarray(x[c * R:(c + 1) * R].T).astype(BF16)
        im = {
            "xT": xT_c, "idx16": idx16, "dstl": d_a, "diso": diso,
            "W1": W1.astype(BF16), "W2": W2.astype(BF16),
        }
        if has_b1:
            im["b1"] = b1.astype(BF16)
        if has_b2:
            im["b2"] = b2.astype(BF16)
        if has_b1 or has_b2:
            invd = np.zeros((1, RP), np.float32)
            invd[0, :R] = 1.0 / dis[c * R:(c + 1) * R]
            im["invd"] = invd.astype(BF16)
        in_maps.append(im)

    res = bass_utils.run_bass_kernel_spmd(
        nc, in_maps, core_ids=list(range(NCORES)), trace=_trace,
        tmpdir=_tmpdir)
    LAST_EXEC_NS = res.exec_time_ns
    LAST_RESULTS = res
    out = np.concatenate([res.results[c]["out"] for c in range(NCORES)], axis=0)
    return out.astype(np.float32)


# revision 11
# speedup vs baseline: 2.4313x; 1.0750x over previous
"""2-layer GCN (PyG GCNConv style) on 8 Trainium2 NeuronCores.

Strategy (graph/node parallel, per sharding hint), v3:
  - Nodes range-sharded across 8 cores (R = N/8 rows each).
  - All matmul operands bf16 (PE 1 cycle/row vs 4 for fp32), fp32 PSUM.
  - Norm factoring: out = dis[dst] * sum_e (dis[src]*xw[src]).  Stored rows
    pre-scaled by dis[src] (ACT-engine scale on the phase-1/3 epilogue),
    output tiles post-scaled by dis[dst] (ACT epilogue).  The per-chunk
    selection matrix S is then pure 0/1 one-hot, built with ONE broadcast
    is_equal per gather batch on DVE.
  - Source-row gathers use gpsimd.dma_gather, batched over GB dst-tiles per
    instruction to amortize the ~1us fixed SWDGE cost.  dma_gather indices
    are int16, so the gather table is split in two halves (row < ha and
    row >= ha) and every (tile, half) gets its own chunks; two gather
    instructions per batch.  Slot i of an instruction lands in
    out[i%128, i//128, :], with idx value at [i%16, 8*chunk + (i%128)//16].
  - Device per core:
      phase 1: xw1 = x_c @ W1, rows scaled by dis  -> AllGather (bf16)
      phase 2+3 fused per dst tile: batched gathers, S one-hot,
               PE-matmul-accumulate S^T @ G, ReLU(dis * ps), PE-transpose
               to h1T, immediately h1 @ W2 (scaled by dis) -> ag2_in
      AllGather (bf16)
      phase 4: same aggregation, out = dis * (S^T @ G2), fp32 out.
  - Host concatenates the 8 row-shards.
"""

import sys

for p in ("/opt/trn_rl_repo",):
    if p not in sys.path:
        sys.path.insert(0, p)

import numpy as np
import ml_dtypes

import concourse.bass as bass
import concourse.bacc as bacc
import concourse.mybir as mybir
import concourse.tile as tile
from concourse import bass_utils
from concourse.masks import make_identity

P = 128
NCORES = 8
GB = 3            # dst tiles per gather batch (shared by both layers)
BF16 = ml_dtypes.bfloat16

PAD_DSTL = 255.0  # is_equal(255, d) is false for every d in 0..127


# ----------------------------------------------------------------------------
# Host-side preprocessing
# ----------------------------------------------------------------------------

def _preprocess(x, edge_index, n_cores, ha=None):
    """Pack per-core edge metadata for the batched dma_gather scheme."""
    N = x.shape[0]
    R = N // n_cores
    assert R * n_cores == N
    ntiles = (R + P - 1) // P

    src = edge_index[0].astype(np.int64)
    dst = edge_index[1].astype(np.int64)
    loops = np.arange(N, dtype=np.int64)
    src = np.concatenate([src, loops])
    dst = np.concatenate([dst, loops])

    deg = np.bincount(dst, minlength=N).astype(np.float64)
    dis = (1.0 / np.sqrt(deg)).astype(np.float32)  # deg>=1 via self-loops

    if ha is None:
        ha = N if N <= 32768 else (N + 1) // 2
    assert ha <= 32768 and (N - ha) <= 32768  # int16 gather indices

    core_id = dst // R
    dloc = dst - core_id * R
    tl = dloc // P
    dstl = (dloc - tl * P).astype(np.float32)
    half = (src >= ha).astype(np.int64)

    key = (core_id * ntiles + tl) * 2 + half
    counts = np.bincount(key, minlength=n_cores * ntiles * 2) \
        .reshape(n_cores, ntiles, 2)
    cmax = counts.max(axis=0)  # [ntiles, 2]
    cA = np.ceil(cmax[:, 0] / P).astype(np.int64)
    cB = np.ceil(cmax[:, 1] / P).astype(np.int64)

    # batch structure: per batch, A-chunks (grouped per tile) then B-chunks
    startA = np.zeros(ntiles, np.int64)
    startB = np.zeros(ntiles, np.int64)
    batches = []
    col = 0
    for t0 in range(0, ntiles, GB):
        t1 = min(t0 + GB, ntiles)
        j0 = col
        for t in range(t0, t1):
            startA[t] = col
            col += cA[t]
        kA = col - j0
        for t in range(t0, t1):
            startB[t] = col
            col += cB[t]
        kB = col - j0 - kA
        batches.append(dict(t0=t0, t1=t1, j0=int(j0), kA=int(kA), kB=int(kB)))
    total = int(col)

    tile_cols = [
        list(range(startA[t], startA[t] + cA[t]))
        + list(range(startB[t], startB[t] + cB[t]))
        for t in range(ntiles)
    ]
    layout = dict(batches=batches, tile_cols=tile_cols, total=total)

    RP = ntiles * P
    packed = []
    for c in range(n_cores):
        m = core_id == c
        s_c = src[m]
        t_c = tl[m]
        d_c = dstl[m]
        h_c = half[m]
        grp = t_c * 2 + h_c
        order = np.argsort(grp, kind="stable")
        s_c, t_c, d_c, h_c, grp = (a[order] for a in (s_c, t_c, d_c, h_c, grp))
        cnt = np.bincount(grp, minlength=ntiles * 2)
        starts = np.cumsum(cnt) - cnt
        pos = np.arange(len(grp)) - np.repeat(starts, cnt)
        start_col = np.where(h_c == 1, startB[t_c], startA[t_c])
        chunk_col = start_col + pos // P
        part = pos % P

        A_dl = np.full((P, total), PAD_DSTL, np.float32)
        A_dl[part, chunk_col] = d_c
        vals = np.where(h_c == 1, s_c - ha, s_c).astype(np.int16)
        idx16 = np.zeros((P, total * 8), np.int16)
        idx16[part % 16, chunk_col * 8 + part // 16] = vals
        # idx block must be replicated across the 8 Q7-core partition stripes
        idx16[16:, :] = np.tile(idx16[0:16, :], (7, 1))

        dis_c = np.zeros(RP, np.float32)
        dis_c[:R] = dis[c * R:(c + 1) * R]
        diso = np.ascontiguousarray(dis_c.reshape(ntiles, P).T)  # [P, ntiles]

        packed.append((idx16, A_dl.astype(BF16), diso))
    return packed, layout, R, ntiles, dis, ha


# ----------------------------------------------------------------------------
# Device kernel builder (parameterized so a tiny config can be sim-tested)
# ----------------------------------------------------------------------------

def build_nc(N, R, ntiles, layout, F0, F1, F2, n_cores, ha,
             has_b1=False, has_b2=False):
    """Build the SPMD Bass program. F0,F1,F2 multiples of 128."""
    f32 = mybir.dt.float32
    bf = mybir.dt.bfloat16
    i16 = mybir.dt.int16
    AF = mybir.ActivationFunctionType
    K0 = F0 // P       # k-tiles in layer-1 matmul
    H1 = F1 // P       # 128-wide blocks of F1
    K2 = F1 // P       # k-tiles in layer-2 matmul (= H1)
    assert F2 <= 512 and F2 % P == 0
    last_rows = R - (ntiles - 1) * P
    RP = ntiles * P    # padded row count
    total = layout["total"]
    batches = layout["batches"]
    tile_cols = layout["tile_cols"]

    nc = bacc.Bacc("TRN2", target_bir_lowering=False, debug=False,
                   num_devices=n_cores, num_swdge_queues=4)

    xT = nc.dram_tensor("xT", [F0, R], bf, kind="ExternalInput").ap()
    idx_d = nc.dram_tensor("idx16", [P, total * 8], i16,
                           kind="ExternalInput").ap()
    dstl_d = nc.dram_tensor("dstl", [P, total], bf, kind="ExternalInput").ap()
    diso_d = nc.dram_tensor("diso", [P, ntiles], f32, kind="ExternalInput").ap()
    W1_d = nc.dram_tensor("W1", [F0, F1], bf, kind="ExternalInput").ap()
    W2_d = nc.dram_tensor("W2", [F1, F2], bf, kind="ExternalInput").ap()
    if has_b1:
        b1_d = nc.dram_tensor("b1", [F1], bf, kind="ExternalInput").ap()
    if has_b2:
        b2_d = nc.dram_tensor("b2", [F2], bf, kind="ExternalInput").ap()
    if has_b1 or has_b2:
        invd_d = nc.dram_tensor("invd", [1, RP], bf, kind="ExternalInput").ap()
    out_d = nc.dram_tensor("out", [R, F2], f32, kind="ExternalOutput").ap()

    rg = [list(range(n_cores))]

    with tile.TileContext(nc) as tc:
        with (
            tc.tile_pool(name="dram", bufs=1, space="DRAM") as dram,
            tc.tile_pool(name="const", bufs=1) as const,
        ):
            ag1_in = dram.tile([R, F1], bf)
            ag1_out = dram.tile([N, F1], bf, addr_space="Shared")
            ag2_in = dram.tile([R, F2], bf)
            ag2_out = dram.tile([N, F2], bf, addr_space="Shared")

            w1_sb = const.tile([P, K0 * F1], bf)
            nc.sync.dma_start(
                out=w1_sb[:].rearrange("p (k f) -> p k f", k=K0),
                in_=W1_d.rearrange("(k p) f -> p k f", p=P))
            w2_sb = const.tile([P, K2 * F2], bf)
            nc.sync.dma_start(
                out=w2_sb[:].rearrange("p (k f) -> p k f", k=K2),
                in_=W2_d.rearrange("(k p) f -> p k f", p=P))
            if has_b1:
                b1_row = const.tile([1, F1], bf)
                nc.sync.dma_start(out=b1_row[:, :], in_=b1_d[None, :])
            if has_b2:
                b2_row = const.tile([1, F2], bf)
                nc.sync.dma_start(out=b2_row[:, :], in_=b2_d[None, :])
            if has_b1 or has_b2:
                invd_sb = const.tile([1, RP], bf)
                nc.sync.dma_start(out=invd_sb[:, :], in_=invd_d[:, :])

            iota_i = const.tile([P, P], mybir.dt.int32)
            nc.gpsimd.iota(iota_i[:], pattern=[[1, P]], base=0,
                           channel_multiplier=0)
            iota_bf = const.tile([P, P], bf)
            nc.vector.tensor_copy(out=iota_bf[:], in_=iota_i[:])
            ident = const.tile([P, P], bf)
            make_identity(nc, ident[:])

            idx_sb = const.tile([P, total * 8], i16)
            nc.sync.dma_start(out=idx_sb[:], in_=idx_d[:])
            dstl_sb = const.tile([P, total], bf)
            nc.sync.dma_start(out=dstl_sb[:], in_=dstl_d[:])
            diso_sb = const.tile([P, ntiles], f32)
            nc.sync.dma_start(out=diso_sb[:], in_=diso_d[:])

            h1T = const.tile([P, H1 * RP], bf)  # h1 transposed, H1 row-blocks

            def build_S(pool, j0, kb, tag):
                """One-hot S[e, c*128+d] = (dstl[e, j0+c] == d) per batch."""
                S = pool.tile([P, kb * P], bf, tag=tag)
                nc.vector.scalar_tensor_tensor(
                    out=S[:].rearrange("p (k d) -> p k d", k=kb),
                    in0=dstl_sb[:, j0:j0 + kb].unsqueeze(2)
                        .broadcast_to([P, kb, P]),
                    scalar=1.0,
                    in1=iota_bf[:].unsqueeze(1).broadcast_to([P, kb, P]),
                    op0=mybir.AluOpType.mult,
                    op1=mybir.AluOpType.is_equal)
                return S

            GMAX = 8  # max chunks (x128 idxs) per dma_gather instruction
            qrr = [0]  # SWDGE queue round-robin across gather instructions

            def gather_batch(pool, b, src_full, F, tag):
                """dma_gathers (per table half, split at GMAX) for one batch."""
                kb = b["kA"] + b["kB"]
                G = pool.tile([P, kb * F], bf, tag=tag,
                              name=f"{tag}_{b['t0']}")
                for (k, coff, lo, hi) in (
                    (b["kA"], 0, 0, ha),
                    (b["kB"], b["kA"], ha, N),
                ):
                    for c0 in range(0, k, GMAX):
                        kk = min(GMAX, k - c0)
                        j = b["j0"] + coff + c0
                        o = coff + c0
                        nc.gpsimd.dma_gather(
                            out_ap=G[:, o * F:(o + kk) * F]
                                .rearrange("p (k f) -> p k f", k=kk),
                            in_ap=src_full[lo:hi, :],
                            idxs_ap=idx_sb[:, 8 * j: 8 * (j + kk)],
                            num_idxs=kk * P,
                            num_idxs_reg=kk * P,
                            elem_size=F,
                            queue_num=qrr[0])
                        qrr[0] = (qrr[0] + 1) % 4
                return G

            # ---------------- phase 1: xw1 = dis * (x_c @ W1) ----------------
            with (
                tc.tile_pool(name="p1x", bufs=1) as p1x,
                tc.tile_pool(name="p1o", bufs=3) as p1o,
                tc.tile_pool(name="p1ps", bufs=2, space="PSUM") as p1ps,
            ):
                xt_sb = p1x.tile([P, K0 * R], bf)
                nc.sync.dma_start(
                    out=xt_sb[:].rearrange("p (k r) -> p k r", k=K0),
                    in_=xT.rearrange("(k p) r -> p k r", p=P))
                for m in range(ntiles):
                    rows = last_rows if m == ntiles - 1 else P
                    ps = p1ps.tile([P, F1], f32)
                    for k in range(K0):
                        nc.tensor.matmul(
                            out=ps[:rows, :],
                            lhsT=xt_sb[:, k * R + m * P: k * R + m * P + rows],
                            rhs=w1_sb[:, k * F1:(k + 1) * F1],
                            start=(k == 0), stop=(k == K0 - 1))
                    os = p1o.tile([P, F1], bf)
                    nc.scalar.activation(out=os[:rows, :], in_=ps[:rows, :],
                                         func=AF.Copy,
                                         scale=diso_sb[:rows, m:m + 1])
                    nc.sync.dma_start(out=ag1_in[m * P: m * P + rows, :],
                                      in_=os[:rows, :])

            nc.gpsimd.collective_compute(
                "AllGather", mybir.AluOpType.bypass, replica_groups=rg,
                ins=[ag1_in[:].opt()], outs=[ag1_out[:].opt()])

            # ------- phase 2 (+3 fused): aggregate layer 1, relu, h1T,
            #         hw2 = h1 @ W2, write ag2_in -------
            with (
                tc.tile_pool(name="p2g", bufs=3) as p2g,
                tc.tile_pool(name="p2s", bufs=3) as p2s,
                tc.tile_pool(name="p2h", bufs=3) as p2h,
                tc.tile_pool(name="p2o", bufs=3) as p2o,
                tc.tile_pool(name="p2ps", bufs=2, space="PSUM") as p2ps,
                tc.tile_pool(name="p2pt", bufs=2, space="PSUM") as p2pt,
                tc.tile_pool(name="p2p3", bufs=2, space="PSUM") as p2p3,
            ):
                for b in batches:
                    j0, kb = b["j0"], b["kA"] + b["kB"]
                    G = gather_batch(p2g, b, ag1_out, F1, "G")
                    S = build_S(p2s, j0, kb, "S")
                    for t in range(b["t0"], b["t1"]):
                        rows = last_rows if t == ntiles - 1 else P
                        cols = tile_cols[t]
                        ps = p2ps.tile([P, F1], f32, tag="ps")
                        for i, ccol in enumerate(cols):
                            rc = ccol - j0
                            nc.tensor.matmul(
                                out=ps[:],
                                lhsT=S[:, rc * P:(rc + 1) * P],
                                rhs=G[:, rc * F1:(rc + 1) * F1],
                                start=(i == 0),
                                stop=(i == len(cols) - 1 and not has_b1))
                        if has_b1:
                            nc.tensor.matmul(
                                out=ps[:], lhsT=invd_sb[:, t * P:(t + 1) * P],
                                rhs=b1_row[:], start=False, stop=True)
                        hm = p2h.tile([P, F1], bf, tag="hm")
                        nc.scalar.activation(out=hm[:], in_=ps[:],
                                             func=AF.Relu,
                                             scale=diso_sb[:, t:t + 1])
                        for h in range(H1):
                            pt = p2pt.tile([P, P], bf, tag="pt")
                            nc.tensor.transpose(
                                out=pt[:], in_=hm[:, h * P:(h + 1) * P],
                                identity=ident[:])
                            nc.scalar.activation(
                                out=h1T[:, h * RP + t * P: h * RP + (t + 1) * P],
                                in_=pt[:], func=AF.Copy)
                        p3 = p2p3.tile([P, F2], f32, tag="p3")
                        for k in range(K2):
                            nc.tensor.matmul(
                                out=p3[:rows, :],
                                lhsT=h1T[:, k * RP + t * P: k * RP + t * P + rows],
                                rhs=w2_sb[:, k * F2:(k + 1) * F2],
                                start=(k == 0), stop=(k == K2 - 1))
                        o3 = p2o.tile([P, F2], bf, tag="o3")
                        nc.scalar.activation(out=o3[:rows, :], in_=p3[:rows, :],
                                             func=AF.Copy,
                                             scale=diso_sb[:rows, t:t + 1])
                        nc.sync.dma_start(out=ag2_in[t * P: t * P + rows, :],
                                          in_=o3[:rows, :])

            nc.gpsimd.collective_compute(
                "AllGather", mybir.AluOpType.bypass, replica_groups=rg,
                ins=[ag2_in[:].opt()], outs=[ag2_out[:].opt()])

            # ------- phase 4: aggregate layer 2, node-major fp32 out -------
            with (
                tc.tile_pool(name="p4g", bufs=3) as p4g,
                tc.tile_pool(name="p4s", bufs=3) as p4s,
                tc.tile_pool(name="p4o", bufs=3) as p4o,
                tc.tile_pool(name="p4ps", bufs=3, space="PSUM") as p4ps,
            ):
                for b in batches:
                    j0, kb = b["j0"], b["kA"] + b["kB"]
                    G2 = gather_batch(p4g, b, ag2_out, F2, "G2")
                    S = build_S(p4s, j0, kb, "S4")
                    for t in range(b["t0"], b["t1"]):
                        rows = last_rows if t == ntiles - 1 else P
                        cols = tile_cols[t]
                        ps = p4ps.tile([P, F2], f32, tag="ps4")
                        for i, ccol in enumerate(cols):
                            rc = ccol - j0
                            nc.tensor.matmul(
                                out=ps[:],
                                lhsT=S[:, rc * P:(rc + 1) * P],
                                rhs=G2[:, rc * F2:(rc + 1) * F2],
                                start=(i == 0),
                                stop=(i == len(cols) - 1 and not has_b2))
                        if has_b2:
                            nc.tensor.matmul(
                                out=ps[:], lhsT=invd_sb[:, t * P:(t + 1) * P],
                                rhs=b2_row[:], start=False, stop=True)
                        ot = p4o.tile([P, F2], f32, tag="ot")
                        nc.scalar.activation(out=ot[:rows, :], in_=ps[:rows, :],
                                             func=AF.Copy,
                                             scale=diso_sb[:rows, t:t + 1])
                        nc.sync.dma_start(out=out_d[t * P: t * P + rows, :],
                                          in_=ot[:rows, :])

    nc.compile()
    return nc


# ----------------------------------------------------------------------------
# Public entry point
# ----------------------------------------------------------------------------

LAST_EXEC_NS = None
LAST_RESULTS = None


def kernel(x, edge_index, W1, b1, W2, b2, _trace=False, _tmpdir=None):
    global LAST_EXEC_NS, LAST_RESULTS
    x = np.asarray(x, np.float32)
    edge_index = np.asarray(edge_index)
    W1 = np.asarray(W1, np.float32)
    b1 = np.asarray(b1, np.float32)
    W2 = np.asarray(W2, np.float32)
    b2 = np.asarray(b2, np.float32)
    N, F0 = x.shape
    F1 = W1.shape[1]
    F2 = W2.shape[1]
    has_b1 = bool(np.any(b1 != 0))
    has_b2 = bool(np.any(b2 != 0))

    packed, layout, R, ntiles, dis, ha = _preprocess(x, edge_index, NCORES)
    nc = build_nc(N, R, ntiles, layout, F0, F1, F2, NCORES, ha,
                  has_b1=has_b1, has_b2=has_b2)

    RP = ntiles * P
    in_maps = []
    for c in range(NCORES):
        idx16, d_a, diso = packed[c]
        xT_c = np.ascontiguousarray(x[c * R:(c + 1) * R].T).astype(BF16)
        im = {
            "xT": xT_c, "idx16": idx16, "dstl": d_a, "diso": diso,
            "W1": W1.astype(BF16), "W2": W2.astype(BF16),
        }
        if has_b1:
            im["b1"] = b1.astype(BF16)
        if has_b2:
            im["b2"] = b2.astype(BF16)
        if has_b1 or has_b2:
            invd = np.zeros((1, RP), np.float32)
            invd[0, :R] = 1.0 / dis[c * R:(c + 1) * R]
            im["invd"] = invd.astype(BF16)
        in_maps.append(im)

    res = bass_utils.run_bass_kernel_spmd(
        nc, in_maps, core_ids=list(range(NCORES)), trace=_trace,
        tmpdir=_tmpdir)
    LAST_EXEC_NS = res.exec_time_ns
    LAST_RESULTS = res
    out = np.concatenate([res.results[c]["out"] for c in range(NCORES)], axis=0)
    return out.astype(np.float32)


# revision 17
# speedup vs baseline: 2.4342x; 1.0012x over previous
"""2-layer GCN (PyG GCNConv style) on 8 Trainium2 NeuronCores.

Strategy (graph/node parallel, per sharding hint), v3:
  - Nodes range-sharded across 8 cores (R = N/8 rows each).
  - All matmul operands bf16 (PE 1 cycle/row vs 4 for fp32), fp32 PSUM.
  - Norm factoring: out = dis[dst] * sum_e (dis[src]*xw[src]).  Stored rows
    pre-scaled by dis[src] (ACT-engine scale on the phase-1/3 epilogue),
    output tiles post-scaled by dis[dst] (ACT epilogue).  The per-chunk
    selection matrix S is then pure 0/1 one-hot, built with ONE broadcast
    is_equal per gather batch on DVE.
  - Source-row gathers use gpsimd.dma_gather, batched over GB dst-tiles per
    instruction to amortize the ~1us fixed SWDGE cost.  dma_gather indices
    are int16, so the gather table is split in two halves (row < ha and
    row >= ha) and every (tile, half) gets its own chunks; two gather
    instructions per batch.  Slot i of an instruction lands in
    out[i%128, i//128, :], with idx value at [i%16, 8*chunk + (i%128)//16].
  - Device per core:
      phase 1: xw1 = x_c @ W1, rows scaled by dis  -> AllGather (bf16)
      phase 2+3 fused per dst tile: batched gathers, S one-hot,
               PE-matmul-accumulate S^T @ G, ReLU(dis * ps), PE-transpose
               to h1T, immediately h1 @ W2 (scaled by dis) -> ag2_in
      AllGather (bf16)
      phase 4: same aggregation, out = dis * (S^T @ G2), fp32 out.
  - Host concatenates the 8 row-shards.
"""

import sys

for p in ("/opt/trn_rl_repo",):
    if p not in sys.path:
        sys.path.insert(0, p)

import numpy as np
import ml_dtypes

import concourse.bass as bass
import concourse.bacc as bacc
import concourse.mybir as mybir
import concourse.tile as tile
from concourse import bass_utils
from concourse.masks import make_identity

P = 128
NCORES = 8
GB = 3            # dst tiles per gather batch (shared by both layers)
BF16 = ml_dtypes.bfloat16

PAD_DSTL = 255.0  # is_equal(255, d) is false for every d in 0..127


# ----------------------------------------------------------------------------
# Host-side preprocessing
# ----------------------------------------------------------------------------

def _preprocess(x, edge_index, n_cores, ha=None):
    """Pack per-core edge metadata for the batched dma_gather scheme."""
    N = x.shape[0]
    R = N // n_cores
    assert R * n_cores == N
    ntiles = (R + P - 1) // P

    src = edge_index[0].astype(np.int64)
    dst = edge_index[1].astype(np.int64)
    loops = np.arange(N, dtype=np.int64)
    src = np.concatenate([src, loops])
    dst = np.concatenate([dst, loops])

    deg = np.bincount(dst, minlength=N).astype(np.float64)
    dis = (1.0 / np.sqrt(deg)).astype(np.float32)  # deg>=1 via self-loops

    if ha is None:
        ha = N if N <= 32768 else (N + 1) // 2
    assert ha <= 32768 and (N - ha) <= 32768  # int16 gather indices

    core_id = dst // R
    dloc = dst - core_id * R
    tl = dloc // P
    dstl = (dloc - tl * P).astype(np.float32)
    half = (src >= ha).astype(np.int64)

    key = (core_id * ntiles + tl) * 2 + half
    counts = np.bincount(key, minlength=n_cores * ntiles * 2) \
        .reshape(n_cores, ntiles, 2)
    cmax = counts.max(axis=0)  # [ntiles, 2]
    cA = np.ceil(cmax[:, 0] / P).astype(np.int64)
    cB = np.ceil(cmax[:, 1] / P).astype(np.int64)

    # batch structure: per batch, A-chunks (grouped per tile) then B-chunks
    startA = np.zeros(ntiles, np.int64)
    startB = np.zeros(ntiles, np.int64)
    batches = []
    col = 0
    for t0 in range(0, ntiles, GB):
        t1 = min(t0 + GB, ntiles)
        j0 = col
        for t in range(t0, t1):
            startA[t] = col
            col += cA[t]
        kA = col - j0
        for t in range(t0, t1):
            startB[t] = col
            col += cB[t]
        kB = col - j0 - kA
        batches.append(dict(t0=t0, t1=t1, j0=int(j0), kA=int(kA), kB=int(kB)))
    total = int(col)

    tile_cols = [
        list(range(startA[t], startA[t] + cA[t]))
        + list(range(startB[t], startB[t] + cB[t]))
        for t in range(ntiles)
    ]
    layout = dict(batches=batches, tile_cols=tile_cols, total=total)

    RP = ntiles * P
    packed = []
    for c in range(n_cores):
        m = core_id == c
        s_c = src[m]
        t_c = tl[m]
        d_c = dstl[m]
        h_c = half[m]
        grp = t_c * 2 + h_c
        order = np.argsort(grp, kind="stable")
        s_c, t_c, d_c, h_c, grp = (a[order] for a in (s_c, t_c, d_c, h_c, grp))
        cnt = np.bincount(grp, minlength=ntiles * 2)
        starts = np.cumsum(cnt) - cnt
        pos = np.arange(len(grp)) - np.repeat(starts, cnt)
        start_col = np.where(h_c == 1, startB[t_c], startA[t_c])
        chunk_col = start_col + pos // P
        part = pos % P

        A_dl = np.full((P, total), PAD_DSTL, np.float32)
        A_dl[part, chunk_col] = d_c
        vals = np.where(h_c == 1, s_c - ha, s_c).astype(np.int16)
        idx16 = np.zeros((P, total * 8), np.int16)
        idx16[part % 16, chunk_col * 8 + part // 16] = vals
        # idx block must be replicated across the 8 Q7-core partition stripes
        idx16[16:, :] = np.tile(idx16[0:16, :], (7, 1))

        dis_c = np.zeros(RP, np.float32)
        dis_c[:R] = dis[c * R:(c + 1) * R]
        diso = np.ascontiguousarray(dis_c.reshape(ntiles, P).T)  # [P, ntiles]

        packed.append((idx16, A_dl.astype(BF16), diso))
    return packed, layout, R, ntiles, dis, ha


# ----------------------------------------------------------------------------
# Device kernel builder (parameterized so a tiny config can be sim-tested)
# ----------------------------------------------------------------------------

def build_nc(N, R, ntiles, layout, F0, F1, F2, n_cores, ha,
             has_b1=False, has_b2=False):
    """Build the SPMD Bass program. F0,F1,F2 multiples of 128."""
    f32 = mybir.dt.float32
    bf = mybir.dt.bfloat16
    i16 = mybir.dt.int16
    AF = mybir.ActivationFunctionType
    K0 = F0 // P       # k-tiles in layer-1 matmul
    H1 = F1 // P       # 128-wide blocks of F1
    K2 = F1 // P       # k-tiles in layer-2 matmul (= H1)
    assert F2 <= 512 and F2 % P == 0
    last_rows = R - (ntiles - 1) * P
    RP = ntiles * P    # padded row count
    total = layout["total"]
    batches = layout["batches"]
    tile_cols = layout["tile_cols"]

    nc = bacc.Bacc("TRN2", target_bir_lowering=False, debug=False,
                   num_devices=n_cores, num_swdge_queues=4)

    xT = nc.dram_tensor("xT", [F0, R], bf, kind="ExternalInput").ap()
    idx_d = nc.dram_tensor("idx16", [P, total * 8], i16,
                           kind="ExternalInput").ap()
    dstl_d = nc.dram_tensor("dstl", [P, total], bf, kind="ExternalInput").ap()
    diso_d = nc.dram_tensor("diso", [P, ntiles], f32, kind="ExternalInput").ap()
    W1_d = nc.dram_tensor("W1", [F0, F1], bf, kind="ExternalInput").ap()
    W2_d = nc.dram_tensor("W2", [F1, F2], bf, kind="ExternalInput").ap()
    if has_b1:
        b1_d = nc.dram_tensor("b1", [F1], bf, kind="ExternalInput").ap()
    if has_b2:
        b2_d = nc.dram_tensor("b2", [F2], bf, kind="ExternalInput").ap()
    if has_b1 or has_b2:
        invd_d = nc.dram_tensor("invd", [1, RP], bf, kind="ExternalInput").ap()
    out_d = nc.dram_tensor("out", [R, F2], f32, kind="ExternalOutput").ap()

    rg = [list(range(n_cores))]

    with tile.TileContext(nc) as tc:
        with (
            tc.tile_pool(name="dram", bufs=1, space="DRAM") as dram,
            tc.tile_pool(name="const", bufs=1) as const,
        ):
            ag1_in = dram.tile([R, F1], bf)
            ag1_out = dram.tile([N, F1], bf, addr_space="Shared")
            ag2_in = dram.tile([R, F2], bf)
            ag2_out = dram.tile([N, F2], bf, addr_space="Shared")

            w1_sb = const.tile([P, K0 * F1], bf)
            nc.sync.dma_start(
                out=w1_sb[:].rearrange("p (k f) -> p k f", k=K0),
                in_=W1_d.rearrange("(k p) f -> p k f", p=P))
            w2_sb = const.tile([P, K2 * F2], bf)
            nc.sync.dma_start(
                out=w2_sb[:].rearrange("p (k f) -> p k f", k=K2),
                in_=W2_d.rearrange("(k p) f -> p k f", p=P))
            if has_b1:
                b1_row = const.tile([1, F1], bf)
                nc.sync.dma_start(out=b1_row[:, :], in_=b1_d[None, :])
            if has_b2:
                b2_row = const.tile([1, F2], bf)
                nc.sync.dma_start(out=b2_row[:, :], in_=b2_d[None, :])
            if has_b1 or has_b2:
                invd_sb = const.tile([1, RP], bf)
                nc.sync.dma_start(out=invd_sb[:, :], in_=invd_d[:, :])

            iota_i = const.tile([P, P], mybir.dt.int32)
            nc.gpsimd.iota(iota_i[:], pattern=[[1, P]], base=0,
                           channel_multiplier=0)
            iota_bf = const.tile([P, P], bf)
            nc.vector.tensor_copy(out=iota_bf[:], in_=iota_i[:])
            ident = const.tile([P, P], bf)
            make_identity(nc, ident[:])

            idx_sb = const.tile([P, total * 8], i16)
            nc.sync.dma_start(out=idx_sb[:], in_=idx_d[:])
            dstl_sb = const.tile([P, total], bf)
            nc.sync.dma_start(out=dstl_sb[:], in_=dstl_d[:])
            diso_sb = const.tile([P, ntiles], f32)
            nc.sync.dma_start(out=diso_sb[:], in_=diso_d[:])

            h1T = const.tile([P, H1 * RP], bf)  # h1 transposed, H1 row-blocks

            def build_S(pool, j0, kb, tag):
                """One-hot S[e, c*128+d] = (dstl[e, j0+c] == d) per batch."""
                S = pool.tile([P, kb * P], bf, tag=tag)
                nc.vector.scalar_tensor_tensor(
                    out=S[:].rearrange("p (k d) -> p k d", k=kb),
                    in0=dstl_sb[:, j0:j0 + kb].unsqueeze(2)
                        .broadcast_to([P, kb, P]),
                    scalar=1.0,
                    in1=iota_bf[:].unsqueeze(1).broadcast_to([P, kb, P]),
                    op0=mybir.AluOpType.mult,
                    op1=mybir.AluOpType.is_equal)
                return S

            GMAX = 8  # max chunks (x128 idxs) per dma_gather instruction
            qrr = [0]  # SWDGE queue round-robin across gather instructions

            def gather_batch(pool, b, src_full, F, tag):
                """dma_gathers (per table half, split at GMAX) for one batch."""
                kb = b["kA"] + b["kB"]
                G = pool.tile([P, kb * F], bf, tag=tag,
                              name=f"{tag}_{b['t0']}")
                for (k, coff, lo, hi) in (
                    (b["kA"], 0, 0, ha),
                    (b["kB"], b["kA"], ha, N),
                ):
                    for c0 in range(0, k, GMAX):
                        kk = min(GMAX, k - c0)
                        j = b["j0"] + coff + c0
                        o = coff + c0
                        nc.gpsimd.dma_gather(
                            out_ap=G[:, o * F:(o + kk) * F]
                                .rearrange("p (k f) -> p k f", k=kk),
                            in_ap=src_full[lo:hi, :],
                            idxs_ap=idx_sb[:, 8 * j: 8 * (j + kk)],
                            num_idxs=kk * P,
                            num_idxs_reg=kk * P,
                            elem_size=F,
                            queue_num=qrr[0])
                        qrr[0] = (qrr[0] + 1) % 4
                return G

            # ---------------- phase 1: xw1 = dis * (x_c @ W1) ----------------
            with (
                tc.tile_pool(name="p1x", bufs=1) as p1x,
                tc.tile_pool(name="p1o", bufs=3) as p1o,
                tc.tile_pool(name="p1ps", bufs=2, space="PSUM") as p1ps,
            ):
                xt_sb = p1x.tile([P, K0 * R], bf)
                nc.sync.dma_start(
                    out=xt_sb[:].rearrange("p (k r) -> p k r", k=K0),
                    in_=xT.rearrange("(k p) r -> p k r", p=P))
                for m in range(ntiles):
                    rows = last_rows if m == ntiles - 1 else P
                    ps = p1ps.tile([P, F1], f32)
                    for k in range(K0):
                        nc.tensor.matmul(
                            out=ps[:rows, :],
                            lhsT=xt_sb[:, k * R + m * P: k * R + m * P + rows],
                            rhs=w1_sb[:, k * F1:(k + 1) * F1],
                            start=(k == 0), stop=(k == K0 - 1))
                    os = p1o.tile([P, F1], bf)
                    nc.scalar.activation(out=os[:rows, :], in_=ps[:rows, :],
                                         func=AF.Copy,
                                         scale=diso_sb[:rows, m:m + 1])
                    nc.sync.dma_start(out=ag1_in[m * P: m * P + rows, :],
                                      in_=os[:rows, :])

            nc.gpsimd.collective_compute(
                "AllGather", mybir.AluOpType.bypass, replica_groups=rg,
                ins=[ag1_in[:].opt()], outs=[ag1_out[:].opt()])

            # ------- phase 2 (+3 fused): aggregate layer 1, relu, h1T,
            #         hw2 = h1 @ W2, write ag2_in -------
            with (
                tc.tile_pool(name="p2g", bufs=3) as p2g,
                tc.tile_pool(name="p2s", bufs=3) as p2s,
                tc.tile_pool(name="p2h", bufs=3) as p2h,
                tc.tile_pool(name="p2o", bufs=3) as p2o,
                tc.tile_pool(name="p2ps", bufs=2, space="PSUM") as p2ps,
                tc.tile_pool(name="p2pt", bufs=2, space="PSUM") as p2pt,
                tc.tile_pool(name="p2p3", bufs=2, space="PSUM") as p2p3,
            ):
                for b in batches:
                    j0, kb = b["j0"], b["kA"] + b["kB"]
                    G = gather_batch(p2g, b, ag1_out, F1, "G")
                    S = build_S(p2s, j0, kb, "S")
                    for t in range(b["t0"], b["t1"]):
                        rows = last_rows if t == ntiles - 1 else P
                        cols = tile_cols[t]
                        ps = p2ps.tile([P, F1], f32, tag="ps")
                        for i, ccol in enumerate(cols):
                            rc = ccol - j0
                            nc.tensor.matmul(
                                out=ps[:],
                                lhsT=S[:, rc * P:(rc + 1) * P],
                                rhs=G[:, rc * F1:(rc + 1) * F1],
                                start=(i == 0),
                                stop=(i == len(cols) - 1 and not has_b1))
                        if has_b1:
                            nc.tensor.matmul(
                                out=ps[:], lhsT=invd_sb[:, t * P:(t + 1) * P],
                                rhs=b1_row[:], start=False, stop=True)
                        hm = p2h.tile([P, F1], bf, tag="hm")
                        nc.scalar.activation(out=hm[:], in_=ps[:],
                                             func=AF.Relu,
                                             scale=diso_sb[:, t:t + 1])
                        for h in range(H1):
                            pt = p2pt.tile([P, P], bf, tag="pt")
                            nc.tensor.transpose(
                                out=pt[:], in_=hm[:, h * P:(h + 1) * P],
                                identity=ident[:])
                            nc.scalar.activation(
                                out=h1T[:, h * RP + t * P: h * RP + (t + 1) * P],
                                in_=pt[:], func=AF.Copy)
                        p3 = p2p3.tile([P, F2], f32, tag="p3")
                        for k in range(K2):
                            nc.tensor.matmul(
                                out=p3[:rows, :],
                                lhsT=h1T[:, k * RP + t * P: k * RP + t * P + rows],
                                rhs=w2_sb[:, k * F2:(k + 1) * F2],
                                start=(k == 0), stop=(k == K2 - 1))
                        o3 = p2o.tile([P, F2], bf, tag="o3")
                        nc.scalar.activation(out=o3[:rows, :], in_=p3[:rows, :],
                                             func=AF.Copy,
                                             scale=diso_sb[:rows, t:t + 1])
                        nc.sync.dma_start(out=ag2_in[t * P: t * P + rows, :],
                                          in_=o3[:rows, :])

            nc.gpsimd.collective_compute(
                "AllGather", mybir.AluOpType.bypass, replica_groups=rg,
                ins=[ag2_in[:].opt()], outs=[ag2_out[:].opt()])

            # ------- phase 4: aggregate layer 2, node-major fp32 out -------
            with (
                tc.tile_pool(name="p4g", bufs=3) as p4g,
                tc.tile_pool(name="p4s", bufs=3) as p4s,
                tc.tile_pool(name="p4o", bufs=3) as p4o,
                tc.tile_pool(name="p4ps", bufs=3, space="PSUM") as p4ps,
            ):
                for b in batches:
                    j0, kb = b["j0"], b["kA"] + b["kB"]
                    G2 = gather_batch(p4g, b, ag2_out, F2, "G2")
                    S = build_S(p4s, j0, kb, "S4")
                    for t in range(b["t0"], b["t1"]):
                        rows = last_rows if t == ntiles - 1 else P
                        cols = tile_cols[t]
                        ps = p4ps.tile([P, F2], f32, tag="ps4")
                        for i, ccol in enumerate(cols):
                            rc = ccol - j0
                            nc.tensor.matmul(
                                out=ps[:],
                                lhsT=S[:, rc * P:(rc + 1) * P],
                                rhs=G2[:, rc * F2:(rc + 1) * F2],
                                start=(i == 0),
                                stop=(i == len(cols) - 1 and not has_b2))
                        if has_b2:
                            nc.tensor.matmul(
                                out=ps[:], lhsT=invd_sb[:, t * P:(t + 1) * P],
                                rhs=b2_row[:], start=False, stop=True)
                        ot = p4o.tile([P, F2], f32, tag="ot")
                        nc.scalar.activation(out=ot[:rows, :], in_=ps[:rows, :],
                                             func=AF.Copy,
                                             scale=diso_sb[:rows, t:t + 1])
                        nc.sync.dma_start(out=out_d[t * P: t * P + rows, :],
                                          in_=ot[:rows, :])

    nc.compile()
    return nc


# ----------------------------------------------------------------------------
# Public entry point
# ----------------------------------------------------------------------------

LAST_EXEC_NS = None
LAST_RESULTS = None


def kernel(x, edge_index, W1, b1, W2, b2, _trace=False, _tmpdir=None):
    global LAST_EXEC_NS, LAST_RESULTS
    x = np.asarray(x, np.float32)
    edge_index = np.asarray(edge_index)
    W1 = np.asarray(W1, np.float32)
    b1 = np.asarray(b1, np.float32)
    W2 = np.asarray(W2, np.float32)
    b2 = np.asarray(b2, np.float32)
    N, F0 = x.shape
    F1 = W1.shape[1]
    F2 = W2.shape[1]
    has_b1 = bool(np.any(b1 != 0))
    has_b2 = bool(np.any(b2 != 0))

    packed, layout, R, ntiles, dis, ha = _preprocess(x, edge_index, NCORES)
    nc = build_nc(N, R, ntiles, layout, F0, F1, F2, NCORES, ha,
                  has_b1=has_b1, has_b2=has_b2)

    RP = ntiles * P
    in_maps = []
    for c in range(NCORES):
        idx16, d_a, diso = packed[c]
        xT_c = np.ascontiguousarray(x[c * R:(c + 1) * R].T).astype(BF16)
        im = {
            "xT": xT_c, "idx16": idx16, "dstl": d_a, "diso": diso,
            "W1": W1.astype(BF16), "W2": W2.astype(BF16),
        }
        if has_b1:
            im["b1"] = b1.astype(BF16)
        if has_b2:
            im["b2"] = b2.astype(BF16)
        if has_b1 or has_b2:
            invd = np.zeros((1, RP), np.float32)
            invd[0, :R] = 1.0 / dis[c * R:(c + 1) * R]
            im["invd"] = invd.astype(BF16)
        in_maps.append(im)

    res = bass_utils.run_bass_kernel_spmd(
        nc, in_maps, core_ids=list(range(NCORES)), trace=_trace,
        tmpdir=_tmpdir)
    LAST_EXEC_NS = res.exec_time_ns
    LAST_RESULTS = res
    out = np.concatenate([res.results[c]["out"] for c in range(NCORES)], axis=0)
    return out.astype(np.float32)
